# revision 8
# baseline (speedup 1.0000x reference)
"""Trainium2 Bass kernel for nn_RNNModelWithConditioning.

Strategy: 8-way model-parallel over the hidden dimension (each core owns a
128-row slice of every gate block), full batch per core (N=256). All
weights stay SBUF-resident across the T=256 recurrence. Host-side numpy
folds the conv + input-side linears + cross-layer links into per-step
matmul-only forms:

  gx1[t] = sum_kt M1[:,kt,:] @ xpad[t+kt]          (conv+Wih0 folded)
  gx2[t] = W2x @ x0[t] + Weff2 @ h1[t] + xc2       (in1/ht0 folded)
  gx3[t] = W3x @ x0[t] + W3h1 @ h1[t] + xc3        (in2/ht1/ht2/ht0 folded)
  y[t]   = F1 @ h1[t] + F2 @ h2[t] + F3 @ h3[t] + yb

Matmul operands are bf16 (weights, x windows, gathered h); the local GRU
state h_own is kept in f32 so rounding does not compound step to step.
The loop is software-pipelined: layer 1 of step t+1 is emitted before
layers 2/3 of step t, so each AllGather's round trip hides under the
other half-step's matmuls. y partials (feature-sliced) accumulate in DRAM
and are ReduceScattered at the end; the host concatenates the 8 chunks.
"""
import numpy as np
import ml_dtypes

import concourse.bass as bass
import concourse.bacc as bacc
import concourse.tile as tile
import concourse.mybir as mybir
from concourse import bass_utils
from concourse.tile_rust import add_dep_helper

B = 256
T = 256
H = 1024
NCORES = 8
SL = 128          # hidden slice per core
NIN = 81
KT = 11           # conv time taps
M3 = 3 * SL       # 384 rows per core (r|z|n)

f32 = mybir.dt.float32
bf16 = mybir.dt.bfloat16
AF = mybir.ActivationFunctionType
bfnp = ml_dtypes.bfloat16

_nc_cache = {}


def _build(t_steps):
    if t_steps in _nc_cache:
        return _nc_cache[t_steps]
    nc = bacc.Bacc("TRN2", target_bir_lowering=False, debug=False,
                   num_devices=NCORES)
    din = {}

    def inp(name, shape, dt=bf16):
        din[name] = nc.dram_tensor(name, shape, dt, kind="ExternalInput").ap()

    inp('xpad', [t_steps + 10, NIN, B])
    inp('m1t', [NIN, KT * M3])
    inp('w2xt', [NIN, M3])
    inp('w3xt', [NIN, M3])
    for nm in ('whh0t', 'weff2t', 'whh1t', 'w3h1t', 'whh2t'):
        inp(nm, [128, 8 * M3])
    inp('ft', [128, 3 * NIN])
    for nm in ('xc1', 'xc2', 'xc3'):
        inp(nm, [128, 3 * B], f32)
    for nm in ('h1t0', 'h2t0', 'h3t0'):
        inp(nm, [128, 8 * B])
    for nm in ('h1own0', 'h2own0', 'h3own0'):
        inp(nm, [128, B], f32)
    ychunk = t_steps * NIN * B // NCORES
    yout = nc.dram_tensor('yout', [ychunk], f32, kind="ExternalOutput").ap()

    with tile.TileContext(nc) as tc:
        with tc.tile_pool(name="wpool", bufs=1) as wp, \
             tc.tile_pool(name="hpool", bufs=2) as hp, \
             tc.tile_pool(name="xwpool", bufs=14) as xp, \
             tc.tile_pool(name="scratch", bufs=1) as sp, \
             tc.tile_pool(name="ownpool", bufs=2) as op_, \
             tc.tile_pool(name="pspool", bufs=1, space="PSUM") as pp, \
             tc.tile_pool(name="drampool", bufs=2, space="DRAM") as dp:

            # ---- load constants ----
            W = {}
            for nm, shape in (('m1t', [NIN, KT * M3]), ('w2xt', [NIN, M3]),
                              ('w3xt', [NIN, M3]), ('whh0t', [128, 8 * M3]),
                              ('weff2t', [128, 8 * M3]), ('whh1t', [128, 8 * M3]),
                              ('w3h1t', [128, 8 * M3]), ('whh2t', [128, 8 * M3]),
                              ('ft', [128, 3 * NIN])):
                w_t = wp.tile(shape, bf16, tag=nm, name=nm + '_sb')
                nc.sync.dma_start(out=w_t[:], in_=din[nm][:])
                W[nm] = w_t
            XC = {}
            for nm in ('xc1', 'xc2', 'xc3'):
                c_t = wp.tile([128, 3 * B], f32, tag=nm, name=nm + '_sb')
                nc.sync.dma_start(out=c_t[:], in_=din[nm][:])
                XC[nm] = c_t

            hT = {}
            for li, nm in ((1, 'h1t0'), (2, 'h2t0'), (3, 'h3t0')):
                h_t = hp.tile([128, 8 * B], bf16, tag=f'h{li}T', name=f'h{li}T_i')
                nc.sync.dma_start(out=h_t[:], in_=din[nm][:])
                hT[li] = h_t
            hown = {}
            for li, nm in ((1, 'h1own0'), (2, 'h2own0'), (3, 'h3own0')):
                h_t = op_.tile([128, B], f32, tag=f'h{li}own', name=f'h{li}own_i')
                nc.sync.dma_start(out=h_t[:], in_=din[nm][:])
                hown[li] = h_t
            hbf = {}   # bf16 copies of own slices (for y matmuls / transport)
            h1gs = {}  # per-step h1 gather tiles (consumed by l23)
            h1bfs = {}  # per-step h1 bf16 slices (consumed by l23's y matmul)

            ypart = dp.tile([t_steps, NIN, B], f32, tag='ypart', name='ypart',
                            bufs=1)

            # ---- x window ring ----
            xw = {}

            def load_xw(j):
                x_t = xp.tile([NIN, B], bf16, tag='xw', name=f'xw{j}')
                nc.sync.dma_start(out=x_t[:], in_=din['xpad'][j])
                xw[j] = x_t

            for j in range(min(13, t_steps + 10)):
                load_xw(j)

            def mmgroup(dst, pairs, first=True, last=True, after=None):
                insts = []
                n = len(pairs)
                for idx, (lh, rh) in enumerate(pairs):
                    bi = nc.tensor.matmul(dst, lh, rh,
                                          start=(first and idx == 0),
                                          stop=(last and idx == n - 1))
                    insts.append(bi)
                if after is not None:
                    add_dep_helper(insts[0].ins, after[-1].ins,
                                   reason="psum bank group order")
                return insts

            def gru_gates(ps_r, ps_z, ps_xn, ps_hn, xc, li, lname):
                """Gate math; updates hown[li] (f32) and hbf[li] (bf16)."""
                tmp = sp.tile([128, B], f32, tag=f'{lname}tmp', name=f'{lname}tmp')
                nc.vector.tensor_add(tmp[:], ps_r, xc[:, :B])
                r = sp.tile([128, B], f32, tag=f'{lname}r', name=f'{lname}r')
                nc.scalar.activation(r[:], tmp[:], AF.Sigmoid)
                tmp2 = sp.tile([128, B], f32, tag=f'{lname}tmp2',
                               name=f'{lname}tmp2')
                nc.vector.tensor_add(tmp2[:], ps_z, xc[:, B:2 * B])
                z = sp.tile([128, B], f32, tag=f'{lname}z', name=f'{lname}z')
                nc.scalar.activation(z[:], tmp2[:], AF.Sigmoid)
                t1 = sp.tile([128, B], f32, tag=f'{lname}t1', name=f'{lname}t1')
                nc.vector.tensor_mul(t1[:], r[:], ps_hn)
                t2 = sp.tile([128, B], f32, tag=f'{lname}t2', name=f'{lname}t2')
                nc.vector.tensor_add(t2[:], t1[:], ps_xn)
                t3 = sp.tile([128, B], f32, tag=f'{lname}t3', name=f'{lname}t3')
                nc.vector.tensor_add(t3[:], t2[:], xc[:, 2 * B:3 * B])
                n_t = sp.tile([128, B], f32, tag=f'{lname}n', name=f'{lname}n')
                nc.scalar.activation(n_t[:], t3[:], AF.Tanh)
                d = sp.tile([128, B], f32, tag=f'{lname}d', name=f'{lname}d')
                nc.vector.tensor_sub(d[:], hown[li][:], n_t[:])
                e = sp.tile([128, B], f32, tag=f'{lname}e', name=f'{lname}e')
                nc.vector.tensor_mul(e[:], z[:], d[:])
                h_new = op_.tile([128, B], f32, tag=f'{lname}own',
                                 name=f'{lname}own')
                nc.vector.tensor_add(h_new[:], n_t[:], e[:])
                hown[li] = h_new
                h_b = op_.tile([128, B], bf16, tag=f'{lname}bf', name=f'{lname}bf')
                nc.scalar.copy(h_b[:], h_new[:])
                hbf[li] = h_b

            def l1(t):
                """Layer-1 of step t: x-side matmuls are emitted before the
                h-side (which waits on AG1(t-1)); produces h1[t] slice + AG1."""
                j = t + 12
                if j < t_steps + 10:
                    load_xw(j)
                psA = pp.tile([128, 2 * B], f32, tag='psA1', name='psA1')
                psB = pp.tile([128, 2 * B], f32, tag='psB1', name='psB1')
                m1, h1c = W['m1t'], hT[1]

                def m1_pairs(g):
                    return [(m1[:, kt * M3 + g * SL: kt * M3 + (g + 1) * SL],
                             xw[t + kt][:]) for kt in range(KT)]

                def whh_pairs(g):
                    return [(W['whh0t'][:, k * M3 + g * SL: k * M3 + (g + 1) * SL],
                             h1c[:, k * B:(k + 1) * B]) for k in range(8)]

                # early (x-only): xn, r-x, z-x
                g_xn = mmgroup(psB[:, :B], m1_pairs(2))
                g_rx = mmgroup(psA[:, :B], m1_pairs(0), last=False)
                g_zx = mmgroup(psB[:, B:2 * B], m1_pairs(1), last=False,
                               after=g_xn)
                # late (need h1 gather): r-h, z-h, hn
                g_rh = mmgroup(psA[:, :B], whh_pairs(0), first=False)
                g_zh = mmgroup(psB[:, B:2 * B], whh_pairs(1), first=False)
                mmgroup(psA[:, B:2 * B], whh_pairs(2), after=g_rh)

                gru_gates(psA[:, :B], psB[:, B:2 * B], psB[:, :B],
                          psA[:, B:2 * B], XC['xc1'], 1, 'L1')

                agin1 = dp.tile([128, B], bf16, tag='agin1', name='agin1', bufs=2)
                nc.sync.dma_start(out=agin1[:], in_=hbf[1][:])
                agout1 = dp.tile([NCORES, 128, B], bf16, tag='agout1',
                                 name='agout1', addr_space="Shared", bufs=2)
                nc.gpsimd.collective_compute(
                    "AllGather", mybir.AluOpType.bypass,
                    replica_groups=[list(range(NCORES))],
                    ins=[agin1[:].opt()], outs=[agout1[:].opt()])
                h1g = hp.tile([128, 8 * B], bf16, tag='h1T', name='h1T')
                for k in range(8):
                    nc.sync.dma_start(out=h1g[:, k * B:(k + 1) * B],
                                      in_=agout1[k])
                hT[1] = h1g
                h1gs[t] = h1g
                h1bfs[t] = hbf[1]

            L23 = (
                (2, 'w2xt', 'whh1t', 'weff2t', 'xc2', 'L2'),
                (3, 'w3xt', 'whh2t', 'w3h1t', 'xc3', 'L3'))

            def l23_head(t, st):
                """Early (AG-independent) matmuls of layers 2/3 of step t:
                w2x + whh parts of the r and z gates."""
                for li, wx, whh, weff, xc, lname in L23:
                    hc = hT[li]
                    psA = pp.tile([128, 2 * B], f32, tag='psA23',
                                  name=f'psA{li}', bufs=2)
                    psB = pp.tile([128, 2 * B], f32, tag='psB23',
                                  name=f'psB{li}', bufs=2)
                    st[li] = (psA, psB)
                    for g, dst in ((0, psA[:, :B]), (1, psB[:, :B])):
                        pairs = [(W[wx][:, g * SL:(g + 1) * SL], xw[t + 5][:])]
                        pairs += [(W[whh][:, k * M3 + g * SL: k * M3 + (g + 1) * SL],
                                   hc[:, k * B:(k + 1) * B]) for k in range(8)]
                        mmgroup(dst, pairs, last=False)

            def l23_tail(t, st):
                """Late matmuls (weff @ h1[t], hn, xn) + gates + AG23 + y."""
                h1c = h1gs.pop(t)
                h1b = h1bfs.pop(t)
                for li, wx, whh, weff, xc, lname in L23:
                    hc = hT[li]
                    psA, psB = st[li]

                    def weff_pairs(g):
                        return [(W[weff][:, k * M3 + g * SL: k * M3 + (g + 1) * SL],
                                 h1c[:, k * B:(k + 1) * B]) for k in range(8)]

                    g_rl = mmgroup(psA[:, :B], weff_pairs(0), first=False)
                    g_zl = mmgroup(psB[:, :B], weff_pairs(1), first=False)
                    mmgroup(psA[:, B:2 * B],
                            [(W[whh][:, k * M3 + 2 * SL: k * M3 + 3 * SL],
                              hc[:, k * B:(k + 1) * B]) for k in range(8)],
                            after=g_rl)
                    mmgroup(psB[:, B:2 * B],
                            [(W[wx][:, 2 * SL:3 * SL], xw[t + 5][:])]
                            + weff_pairs(2), after=g_zl)
                    gru_gates(psA[:, :B], psB[:, :B], psB[:, B:2 * B],
                              psA[:, B:2 * B], XC[xc], li, lname)

                # AllGather h2 & h3 (skip after last step)
                if t + 1 < t_steps:
                    agin23 = dp.tile([2, 128, B], bf16, tag='agin23',
                                     name='agin23', bufs=2)
                    nc.sync.dma_start(out=agin23[0], in_=hbf[2][:])
                    nc.sync.dma_start(out=agin23[1], in_=hbf[3][:])
                    agout23 = dp.tile([NCORES, 2, 128, B], bf16, tag='agout23',
                                      name='agout23', addr_space="Shared", bufs=2)
                    nc.gpsimd.collective_compute(
                        "AllGather", mybir.AluOpType.bypass,
                        replica_groups=[list(range(NCORES))],
                        ins=[agin23[:].opt()], outs=[agout23[:].opt()])
                    h2g = hp.tile([128, 8 * B], bf16, tag='h2T', name='h2T')
                    h3g = hp.tile([128, 8 * B], bf16, tag='h3T', name='h3T')
                    for k in range(0, 8, 2):
                        nc.sync.dma_start(
                            out=h2g[:, k * B:(k + 2) * B],
                            in_=agout23[k:k + 2, 0].rearrange("k p b -> p k b"))
                        nc.sync.dma_start(
                            out=h3g[:, k * B:(k + 2) * B],
                            in_=agout23[k:k + 2, 1].rearrange("k p b -> p k b"))
                    hT[2], hT[3] = h2g, h3g

                # y partials from own slices
                ps_y = pp.tile([NIN, B], f32, tag='yps', name='yps')
                nc.tensor.matmul(ps_y[:], W['ft'][:, 0:NIN], h1b[:],
                                 start=True, stop=False)
                nc.tensor.matmul(ps_y[:], W['ft'][:, NIN:2 * NIN], hbf[2][:],
                                 start=False, stop=False)
                nc.tensor.matmul(ps_y[:], W['ft'][:, 2 * NIN:3 * NIN], hbf[3][:],
                                 start=False, stop=True)
                ysb = sp.tile([NIN, B], f32, tag='ysb', name='ysb')
                nc.vector.tensor_copy(ysb[:], ps_y[:])
                nc.sync.dma_start(out=ypart[t], in_=ysb[:])
                xw.pop(t - 1, None)

            # ---- software-pipelined loop ----
            l1(0)
            for t in range(t_steps):
                st = {}
                l23_head(t, st)
                if t + 1 < t_steps:
                    l1(t + 1)
                l23_tail(t, st)

            # ---- final ReduceScatter of y partials ----
            yred = dp.tile([t_steps * NIN * B // NCORES], f32, tag='yred',
                           name='yred', bufs=1)
            nc.gpsimd.collective_compute(
                "ReduceScatter", mybir.AluOpType.add,
                replica_groups=[list(range(NCORES))],
                ins=[ypart[:].opt()], outs=[yred[:].opt()])
            nc.sync.dma_start(out=yout[:], in_=yred[:])

    nc.compile()
    _nc_cache[t_steps] = nc
    return nc


def _prepare(x, cond, h1, h2, h3, params, t_steps):
    """Host-side folding. Returns (in_maps, yb)."""
    p = params
    fp = np.float32

    def A(v):
        return np.ascontiguousarray(np.asarray(v), dtype=fp)

    def BF(v):
        return np.ascontiguousarray(np.asarray(v, dtype=fp)).astype(bfnp)

    Wih0, Whh0 = A(p['Wih0']), A(p['Whh0'])
    Wih1, Whh1 = A(p['Wih1']), A(p['Whh1'])
    Wih2, Whh2 = A(p['Wih2']), A(p['Whh2'])
    conv_w = A(p['conv_w'])[:, 0]          # [64, 21, 11]
    conv_b = A(p['conv_b'])
    cond_np = A(cond)

    cond1 = cond_np @ A(p['cond0_w']).T + A(p['cond0_b'])
    cond2 = cond_np @ A(p['cond1_w']).T + A(p['cond1_b'])
    cond3 = cond_np @ A(p['cond2_w']).T + A(p['cond2_b'])

    # --- M1 fold: conv + Wih0 ---
    Wr = Wih0[:, :1984].reshape(3 * H, 64, 31)
    M1 = np.zeros((3 * H, KT, NIN), np.float32)
    for kf in range(21):
        tmp = np.einsum('rcf,ck->rfk', Wr, conv_w[:, kf, :], optimize=True)
        fins = 2 * np.arange(31) + kf
        M1[:, :, fins] += tmp.transpose(0, 2, 1)
    c1 = Wih0[:, :1984] @ np.repeat(conv_b, 31)
    xc1 = Wih0[:, 1984:] @ cond1.T + c1[:, None]

    W2x = Wih1[:, :H] @ A(p['in1_w'])
    Weff2 = Wih1[:, :H] @ A(p['ht0_w'])
    xc2 = (Wih1[:, :H] @ (A(p['in1_b']) + A(p['ht0_b'])))[:, None] \
        + Wih1[:, H:] @ cond2.T

    W3x = Wih2[:, :H] @ A(p['in2_w'])
    W3h1 = Wih2[:, :H] @ (A(p['ht1_w']) + A(p['ht2_w']) @ A(p['ht0_w']))
    xc3 = (Wih2[:, :H] @ (A(p['in2_b']) + A(p['ht1_b']) + A(p['ht2_b'])
                          + A(p['ht2_w']) @ A(p['ht0_b'])))[:, None] \
        + Wih2[:, H:] @ cond3.T

    F1 = A(p['final_w']) @ A(p['out0_w'])
    F2 = A(p['final_w']) @ A(p['out1_w'])
    F3 = A(p['final_w']) @ A(p['out2_w'])
    yb = A(p['final_w']) @ (A(p['out0_b']) + A(p['out1_b'])
                            + A(p['out2_b'])) + A(p['final_b'])

    # --- x: [B,1,81,T] -> padded [T+10, 81, B] ---
    xs = A(x)[:, 0, :, :t_steps]                      # [B, 81, t]
    xpad = np.zeros((t_steps + 10, NIN, B), np.float32)
    xpad[5:5 + t_steps] = xs.transpose(2, 1, 0)
    xpad = xpad.astype(bfnp)

    h1T = A(h1).T.reshape(8, 128, B).transpose(1, 0, 2).reshape(128, 8 * B)
    h2T = A(h2).T.reshape(8, 128, B).transpose(1, 0, 2).reshape(128, 8 * B)
    h3T = A(h3).T.reshape(8, 128, B).transpose(1, 0, 2).reshape(128, 8 * B)
    h1T, h2T, h3T = h1T.astype(bfnp), h2T.astype(bfnp), h3T.astype(bfnp)

    def kmaj(w):          # [384, 1024] -> lhsT sbuf layout [128, 8*384]
        return np.ascontiguousarray(
            w.T.reshape(8, 128, M3).transpose(1, 0, 2).reshape(128, 8 * M3)
        ).astype(bfnp)

    in_maps = []
    for i in range(NCORES):
        idx = np.concatenate([np.arange(g * H + i * SL, g * H + (i + 1) * SL)
                              for g in range(3)])
        m1t = np.ascontiguousarray(
            M1[idx].transpose(2, 1, 0).reshape(NIN, KT * M3)).astype(bfnp)
        im = {
            'xpad': xpad,
            'm1t': m1t,
            'w2xt': BF(W2x[idx].T),
            'w3xt': BF(W3x[idx].T),
            'whh0t': kmaj(Whh0[idx]),
            'weff2t': kmaj(Weff2[idx]),
            'whh1t': kmaj(Whh1[idx]),
            'w3h1t': kmaj(W3h1[idx]),
            'whh2t': kmaj(Whh2[idx]),
            'ft': BF(np.concatenate(
                [F1[:, i * SL:(i + 1) * SL].T,
                 F2[:, i * SL:(i + 1) * SL].T,
                 F3[:, i * SL:(i + 1) * SL].T], axis=1)),
            'xc1': np.ascontiguousarray(
                xc1[idx].reshape(3, SL, B).transpose(1, 0, 2).reshape(SL, 3 * B)),
            'xc2': np.ascontiguousarray(
                xc2[idx].reshape(3, SL, B).transpose(1, 0, 2).reshape(SL, 3 * B)),
            'xc3': np.ascontiguousarray(
                xc3[idx].reshape(3, SL, B).transpose(1, 0, 2).reshape(SL, 3 * B)),
            'h1t0': h1T, 'h2t0': h2T, 'h3t0': h3T,
            'h1own0': np.ascontiguousarray(A(h1).T[i * SL:(i + 1) * SL]),
            'h2own0': np.ascontiguousarray(A(h2).T[i * SL:(i + 1) * SL]),
            'h3own0': np.ascontiguousarray(A(h3).T[i * SL:(i + 1) * SL]),
        }
        in_maps.append(im)
    return in_maps, yb


def _run(x, cond, h1, h2, h3, params, t_steps=T, trace=False):
    nc = _build(t_steps)
    in_maps, yb = _prepare(x, cond, h1, h2, h3, params, t_steps)
    res = bass_utils.run_bass_kernel_spmd(
        nc, in_maps, core_ids=list(range(NCORES)), trace=trace)
    chunks = [res.results[i]['yout'] for i in range(NCORES)]
    y = np.concatenate(chunks).reshape(t_steps, NIN, B).transpose(0, 2, 1)
    y = y + yb[None, None, :]
    return np.ascontiguousarray(y, dtype=np.float32), res


def kernel(x, cond, h1, h2, h3, params):
    y, _ = _run(x, cond, h1, h2, h3, params)
    return y


# revision 9
# speedup vs baseline: 1.0072x; 1.0072x over previous
"""Trainium2 Bass kernel for nn_RNNModelWithConditioning.

Strategy: 8-way model-parallel over the hidden dimension (each core owns a
128-row slice of every gate block), full batch per core (N=256). All
weights stay SBUF-resident across the T=256 recurrence. Host-side numpy
folds the conv + input-side linears + cross-layer links into per-step
matmul-only forms:

  gx1[t] = sum_kt M1[:,kt,:] @ xpad[t+kt]          (conv+Wih0 folded)
  gx2[t] = W2x @ x0[t] + Weff2 @ h1[t] + xc2       (in1/ht0 folded)
  gx3[t] = W3x @ x0[t] + W3h1 @ h1[t] + xc3        (in2/ht1/ht2/ht0 folded)
  y[t]   = F1 @ h1[t] + F2 @ h2[t] + F3 @ h3[t] + yb

Matmul operands are bf16 (weights, x windows, gathered h); the local GRU
state h_own is kept in f32 so rounding does not compound step to step.
The loop is software-pipelined: layer 1 of step t+1 is emitted before
layers 2/3 of step t, so each AllGather's round trip hides under the
other half-step's matmuls. y partials (feature-sliced) accumulate in DRAM
and are ReduceScattered at the end; the host concatenates the 8 chunks.
"""
import numpy as np
import ml_dtypes

import concourse.bass as bass
import concourse.bacc as bacc
import concourse.tile as tile
import concourse.mybir as mybir
from concourse import bass_utils
from concourse.tile_rust import add_dep_helper

B = 256
T = 256
H = 1024
NCORES = 8
SL = 128          # hidden slice per core
NIN = 81
KT = 11           # conv time taps
M3 = 3 * SL       # 384 rows per core (r|z|n)

f32 = mybir.dt.float32
bf16 = mybir.dt.bfloat16
AF = mybir.ActivationFunctionType
bfnp = ml_dtypes.bfloat16

_nc_cache = {}


def _build(t_steps):
    if t_steps in _nc_cache:
        return _nc_cache[t_steps]
    nc = bacc.Bacc("TRN2", target_bir_lowering=False, debug=False,
                   num_devices=NCORES)
    din = {}

    def inp(name, shape, dt=bf16):
        din[name] = nc.dram_tensor(name, shape, dt, kind="ExternalInput").ap()

    inp('xpad', [t_steps + 10, NIN, B])
    inp('m1t', [NIN, KT * M3])
    inp('w2xt', [NIN, M3])
    inp('w3xt', [NIN, M3])
    for nm in ('whh0t', 'weff2t', 'whh1t', 'w3h1t', 'whh2t'):
        inp(nm, [128, 8 * M3])
    inp('ft', [128, 3 * NIN])
    for nm in ('xc1', 'xc2', 'xc3'):
        inp(nm, [128, 3 * B], f32)
    for nm in ('h1t0', 'h2t0', 'h3t0'):
        inp(nm, [128, 8 * B])
    for nm in ('h1own0', 'h2own0', 'h3own0'):
        inp(nm, [128, B], f32)
    ychunk = t_steps * NIN * B // NCORES
    yout = nc.dram_tensor('yout', [ychunk], f32, kind="ExternalOutput").ap()

    with tile.TileContext(nc) as tc:
        with tc.tile_pool(name="wpool", bufs=1) as wp, \
             tc.tile_pool(name="hpool", bufs=2) as hp, \
             tc.tile_pool(name="xwpool", bufs=14) as xp, \
             tc.tile_pool(name="scratch", bufs=1) as sp, \
             tc.tile_pool(name="ownpool", bufs=2) as op_, \
             tc.tile_pool(name="pspool", bufs=1, space="PSUM") as pp, \
             tc.tile_pool(name="drampool", bufs=2, space="DRAM") as dp:

            # ---- load constants ----
            W = {}
            for nm, shape in (('m1t', [NIN, KT * M3]), ('w2xt', [NIN, M3]),
                              ('w3xt', [NIN, M3]), ('whh0t', [128, 8 * M3]),
                              ('weff2t', [128, 8 * M3]), ('whh1t', [128, 8 * M3]),
                              ('w3h1t', [128, 8 * M3]), ('whh2t', [128, 8 * M3]),
                              ('ft', [128, 3 * NIN])):
                w_t = wp.tile(shape, bf16, tag=nm, name=nm + '_sb')
                nc.sync.dma_start(out=w_t[:], in_=din[nm][:])
                W[nm] = w_t
            XC = {}
            for nm in ('xc1', 'xc2', 'xc3'):
                c_t = wp.tile([128, 3 * B], f32, tag=nm, name=nm + '_sb')
                nc.sync.dma_start(out=c_t[:], in_=din[nm][:])
                XC[nm] = c_t

            hT = {}
            for li, nm in ((1, 'h1t0'), (2, 'h2t0'), (3, 'h3t0')):
                h_t = hp.tile([128, 8 * B], bf16, tag=f'h{li}T', name=f'h{li}T_i')
                nc.sync.dma_start(out=h_t[:], in_=din[nm][:])
                hT[li] = h_t
            hown = {}
            for li, nm in ((1, 'h1own0'), (2, 'h2own0'), (3, 'h3own0')):
                h_t = op_.tile([128, B], f32, tag=f'h{li}own', name=f'h{li}own_i')
                nc.sync.dma_start(out=h_t[:], in_=din[nm][:])
                hown[li] = h_t
            hbf = {}   # bf16 copies of own slices (for y matmuls / transport)
            h1gs = {}  # per-step h1 gather tiles (consumed by l23)
            h1bfs = {}  # per-step h1 bf16 slices (consumed by l23's y matmul)

            ypart = dp.tile([t_steps, NIN, B], f32, tag='ypart', name='ypart',
                            bufs=1)

            # ---- x window ring ----
            xw = {}

            def load_xw(j):
                x_t = xp.tile([NIN, B], bf16, tag='xw', name=f'xw{j}')
                nc.sync.dma_start(out=x_t[:], in_=din['xpad'][j])
                xw[j] = x_t

            for j in range(min(13, t_steps + 10)):
                load_xw(j)

            def mmgroup(dst, pairs, first=True, last=True, after=None):
                insts = []
                n = len(pairs)
                for idx, (lh, rh) in enumerate(pairs):
                    bi = nc.tensor.matmul(dst, lh, rh,
                                          start=(first and idx == 0),
                                          stop=(last and idx == n - 1))
                    insts.append(bi)
                if after is not None:
                    add_dep_helper(insts[0].ins, after[-1].ins,
                                   reason="psum bank group order")
                return insts

            def gru_gates(ps_r, ps_z, ps_xn, ps_hn, xc, li, lname):
                """Gate math; updates hown[li] (f32) and hbf[li] (bf16)."""
                tmp = sp.tile([128, B], f32, tag=f'{lname}tmp', name=f'{lname}tmp')
                nc.vector.tensor_add(tmp[:], ps_r, xc[:, :B])
                r = sp.tile([128, B], f32, tag=f'{lname}r', name=f'{lname}r')
                nc.scalar.activation(r[:], tmp[:], AF.Sigmoid)
                tmp2 = sp.tile([128, B], f32, tag=f'{lname}tmp2',
                               name=f'{lname}tmp2')
                nc.vector.tensor_add(tmp2[:], ps_z, xc[:, B:2 * B])
                z = sp.tile([128, B], f32, tag=f'{lname}z', name=f'{lname}z')
                nc.scalar.activation(z[:], tmp2[:], AF.Sigmoid)
                t1 = sp.tile([128, B], f32, tag=f'{lname}t1', name=f'{lname}t1')
                nc.vector.tensor_mul(t1[:], r[:], ps_hn)
                t2 = sp.tile([128, B], f32, tag=f'{lname}t2', name=f'{lname}t2')
                nc.vector.tensor_add(t2[:], t1[:], ps_xn)
                t3 = sp.tile([128, B], f32, tag=f'{lname}t3', name=f'{lname}t3')
                nc.vector.tensor_add(t3[:], t2[:], xc[:, 2 * B:3 * B])
                n_t = sp.tile([128, B], f32, tag=f'{lname}n', name=f'{lname}n')
                nc.scalar.activation(n_t[:], t3[:], AF.Tanh)
                d = sp.tile([128, B], f32, tag=f'{lname}d', name=f'{lname}d')
                nc.vector.tensor_sub(d[:], hown[li][:], n_t[:])
                e = sp.tile([128, B], f32, tag=f'{lname}e', name=f'{lname}e')
                nc.vector.tensor_mul(e[:], z[:], d[:])
                h_new = op_.tile([128, B], f32, tag=f'{lname}own',
                                 name=f'{lname}own')
                nc.vector.tensor_add(h_new[:], n_t[:], e[:])
                hown[li] = h_new
                h_b = op_.tile([128, B], bf16, tag=f'{lname}bf', name=f'{lname}bf')
                nc.scalar.copy(h_b[:], h_new[:])
                hbf[li] = h_b

            def l1(t):
                """Layer-1 of step t: x-side matmuls are emitted before the
                h-side (which waits on AG1(t-1)); produces h1[t] slice + AG1."""
                j = t + 12
                if j < t_steps + 10:
                    load_xw(j)
                psA = pp.tile([128, 2 * B], f32, tag='psA1', name='psA1')
                psB = pp.tile([128, 2 * B], f32, tag='psB1', name='psB1')
                m1, h1c = W['m1t'], hT[1]

                def m1_pairs(g):
                    return [(m1[:, kt * M3 + g * SL: kt * M3 + (g + 1) * SL],
                             xw[t + kt][:]) for kt in range(KT)]

                def whh_pairs(g):
                    return [(W['whh0t'][:, k * M3 + g * SL: k * M3 + (g + 1) * SL],
                             h1c[:, k * B:(k + 1) * B]) for k in range(8)]

                # early (x-only): xn, r-x, z-x
                g_xn = mmgroup(psB[:, :B], m1_pairs(2))
                g_rx = mmgroup(psA[:, :B], m1_pairs(0), last=False)
                g_zx = mmgroup(psB[:, B:2 * B], m1_pairs(1), last=False,
                               after=g_xn)
                # late (need h1 gather): r-h, z-h, hn
                g_rh = mmgroup(psA[:, :B], whh_pairs(0), first=False)
                g_zh = mmgroup(psB[:, B:2 * B], whh_pairs(1), first=False)
                mmgroup(psA[:, B:2 * B], whh_pairs(2), after=g_rh)

                gru_gates(psA[:, :B], psB[:, B:2 * B], psB[:, :B],
                          psA[:, B:2 * B], XC['xc1'], 1, 'L1')

                agin1 = dp.tile([128, B], bf16, tag='agin1', name='agin1', bufs=2)
                nc.sync.dma_start(out=agin1[:], in_=hbf[1][:])
                agout1 = dp.tile([NCORES, 128, B], bf16, tag='agout1',
                                 name='agout1', addr_space="Shared", bufs=2)
                nc.gpsimd.collective_compute(
                    "AllGather", mybir.AluOpType.bypass,
                    replica_groups=[list(range(NCORES))],
                    ins=[agin1[:].opt()], outs=[agout1[:].opt()])
                h1g = hp.tile([128, 8 * B], bf16, tag='h1T', name='h1T')
                for k in range(8):
                    nc.sync.dma_start(out=h1g[:, k * B:(k + 1) * B],
                                      in_=agout1[k])
                hT[1] = h1g
                h1gs[t] = h1g
                h1bfs[t] = hbf[1]

            L23 = (
                (2, 'w2xt', 'whh1t', 'weff2t', 'xc2', 'L2'),
                (3, 'w3xt', 'whh2t', 'w3h1t', 'xc3', 'L3'))

            def l23_head(t, st):
                """Early (AG-independent) matmuls of layers 2/3 of step t:
                w2x + whh parts of the r and z gates."""
                for li, wx, whh, weff, xc, lname in L23:
                    hc = hT[li]
                    psA = pp.tile([128, 2 * B], f32, tag='psA23',
                                  name=f'psA{li}', bufs=2)
                    psB = pp.tile([128, 2 * B], f32, tag='psB23',
                                  name=f'psB{li}', bufs=2)
                    st[li] = (psA, psB)
                    for g, dst in ((0, psA[:, :B]), (1, psB[:, :B])):
                        pairs = [(W[wx][:, g * SL:(g + 1) * SL], xw[t + 5][:])]
                        pairs += [(W[whh][:, k * M3 + g * SL: k * M3 + (g + 1) * SL],
                                   hc[:, k * B:(k + 1) * B]) for k in range(8)]
                        mmgroup(dst, pairs, last=False)

            def l23_tail(t, st):
                """Late matmuls (weff @ h1[t], hn, xn) + gates + AG23 + y."""
                h1c = h1gs.pop(t)
                h1b = h1bfs.pop(t)
                for li, wx, whh, weff, xc, lname in L23:
                    hc = hT[li]
                    psA, psB = st[li]

                    def weff_pairs(g):
                        return [(W[weff][:, k * M3 + g * SL: k * M3 + (g + 1) * SL],
                                 h1c[:, k * B:(k + 1) * B]) for k in range(8)]

                    g_rl = mmgroup(psA[:, :B], weff_pairs(0), first=False)
                    g_zl = mmgroup(psB[:, :B], weff_pairs(1), first=False)
                    mmgroup(psA[:, B:2 * B],
                            [(W[whh][:, k * M3 + 2 * SL: k * M3 + 3 * SL],
                              hc[:, k * B:(k + 1) * B]) for k in range(8)],
                            after=g_rl)
                    mmgroup(psB[:, B:2 * B],
                            [(W[wx][:, 2 * SL:3 * SL], xw[t + 5][:])]
                            + weff_pairs(2), after=g_zl)
                    gru_gates(psA[:, :B], psB[:, :B], psB[:, B:2 * B],
                              psA[:, B:2 * B], XC[xc], li, lname)

                # AllGather h2 & h3 (skip after last step)
                if t + 1 < t_steps:
                    agin23 = dp.tile([2, 128, B], bf16, tag='agin23',
                                     name='agin23', bufs=2)
                    nc.sync.dma_start(out=agin23[0], in_=hbf[2][:])
                    nc.sync.dma_start(out=agin23[1], in_=hbf[3][:])
                    agout23 = dp.tile([NCORES, 2, 128, B], bf16, tag='agout23',
                                      name='agout23', addr_space="Shared", bufs=2)
                    nc.gpsimd.collective_compute(
                        "AllGather", mybir.AluOpType.bypass,
                        replica_groups=[list(range(NCORES))],
                        ins=[agin23[:].opt()], outs=[agout23[:].opt()])
                    h2g = hp.tile([128, 8 * B], bf16, tag='h2T', name='h2T')
                    h3g = hp.tile([128, 8 * B], bf16, tag='h3T', name='h3T')
                    for k in range(0, 8, 2):
                        nc.sync.dma_start(
                            out=h2g[:, k * B:(k + 2) * B],
                            in_=agout23[k:k + 2, 0].rearrange("k p b -> p k b"))
                        nc.sync.dma_start(
                            out=h3g[:, k * B:(k + 2) * B],
                            in_=agout23[k:k + 2, 1].rearrange("k p b -> p k b"))
                    hT[2], hT[3] = h2g, h3g

                # y partials from own slices
                ps_y = pp.tile([NIN, B], f32, tag='yps', name='yps')
                nc.tensor.matmul(ps_y[:], W['ft'][:, 0:NIN], h1b[:],
                                 start=True, stop=False)
                nc.tensor.matmul(ps_y[:], W['ft'][:, NIN:2 * NIN], hbf[2][:],
                                 start=False, stop=False)
                nc.tensor.matmul(ps_y[:], W['ft'][:, 2 * NIN:3 * NIN], hbf[3][:],
                                 start=False, stop=True)
                ysb = sp.tile([NIN, B], f32, tag='ysb', name='ysb')
                nc.vector.tensor_copy(ysb[:], ps_y[:])
                nc.sync.dma_start(out=ypart[t], in_=ysb[:])
                xw.pop(t - 1, None)

            # ---- software-pipelined loop ----
            l1(0)
            for t in range(t_steps):
                st = {}
                if t + 1 < t_steps:
                    l1(t + 1)
                l23_head(t, st)
                l23_tail(t, st)

            # ---- final ReduceScatter of y partials ----
            yred = dp.tile([t_steps * NIN * B // NCORES], f32, tag='yred',
                           name='yred', bufs=1)
            nc.gpsimd.collective_compute(
                "ReduceScatter", mybir.AluOpType.add,
                replica_groups=[list(range(NCORES))],
                ins=[ypart[:].opt()], outs=[yred[:].opt()])
            nc.sync.dma_start(out=yout[:], in_=yred[:])

    nc.compile()
    _nc_cache[t_steps] = nc
    return nc


def _prepare(x, cond, h1, h2, h3, params, t_steps):
    """Host-side folding. Returns (in_maps, yb)."""
    p = params
    fp = np.float32

    def A(v):
        return np.ascontiguousarray(np.asarray(v), dtype=fp)

    def BF(v):
        return np.ascontiguousarray(np.asarray(v, dtype=fp)).astype(bfnp)

    Wih0, Whh0 = A(p['Wih0']), A(p['Whh0'])
    Wih1, Whh1 = A(p['Wih1']), A(p['Whh1'])
    Wih2, Whh2 = A(p['Wih2']), A(p['Whh2'])
    conv_w = A(p['conv_w'])[:, 0]          # [64, 21, 11]
    conv_b = A(p['conv_b'])
    cond_np = A(cond)

    cond1 = cond_np @ A(p['cond0_w']).T + A(p['cond0_b'])
    cond2 = cond_np @ A(p['cond1_w']).T + A(p['cond1_b'])
    cond3 = cond_np @ A(p['cond2_w']).T + A(p['cond2_b'])

    # --- M1 fold: conv + Wih0 ---
    Wr = Wih0[:, :1984].reshape(3 * H, 64, 31)
    M1 = np.zeros((3 * H, KT, NIN), np.float32)
    for kf in range(21):
        tmp = np.einsum('rcf,ck->rfk', Wr, conv_w[:, kf, :], optimize=True)
        fins = 2 * np.arange(31) + kf
        M1[:, :, fins] += tmp.transpose(0, 2, 1)
    c1 = Wih0[:, :1984] @ np.repeat(conv_b, 31)
    xc1 = Wih0[:, 1984:] @ cond1.T + c1[:, None]

    W2x = Wih1[:, :H] @ A(p['in1_w'])
    Weff2 = Wih1[:, :H] @ A(p['ht0_w'])
    xc2 = (Wih1[:, :H] @ (A(p['in1_b']) + A(p['ht0_b'])))[:, None] \
        + Wih1[:, H:] @ cond2.T

    W3x = Wih2[:, :H] @ A(p['in2_w'])
    W3h1 = Wih2[:, :H] @ (A(p['ht1_w']) + A(p['ht2_w']) @ A(p['ht0_w']))
    xc3 = (Wih2[:, :H] @ (A(p['in2_b']) + A(p['ht1_b']) + A(p['ht2_b'])
                          + A(p['ht2_w']) @ A(p['ht0_b'])))[:, None] \
        + Wih2[:, H:] @ cond3.T

    F1 = A(p['final_w']) @ A(p['out0_w'])
    F2 = A(p['final_w']) @ A(p['out1_w'])
    F3 = A(p['final_w']) @ A(p['out2_w'])
    yb = A(p['final_w']) @ (A(p['out0_b']) + A(p['out1_b'])
                            + A(p['out2_b'])) + A(p['final_b'])

    # --- x: [B,1,81,T] -> padded [T+10, 81, B] ---
    xs = A(x)[:, 0, :, :t_steps]                      # [B, 81, t]
    xpad = np.zeros((t_steps + 10, NIN, B), np.float32)
    xpad[5:5 + t_steps] = xs.transpose(2, 1, 0)
    xpad = xpad.astype(bfnp)

    h1T = A(h1).T.reshape(8, 128, B).transpose(1, 0, 2).reshape(128, 8 * B)
    h2T = A(h2).T.reshape(8, 128, B).transpose(1, 0, 2).reshape(128, 8 * B)
    h3T = A(h3).T.reshape(8, 128, B).transpose(1, 0, 2).reshape(128, 8 * B)
    h1T, h2T, h3T = h1T.astype(bfnp), h2T.astype(bfnp), h3T.astype(bfnp)

    def kmaj(w):          # [384, 1024] -> lhsT sbuf layout [128, 8*384]
        return np.ascontiguousarray(
            w.T.reshape(8, 128, M3).transpose(1, 0, 2).reshape(128, 8 * M3)
        ).astype(bfnp)

    in_maps = []
    for i in range(NCORES):
        idx = np.concatenate([np.arange(g * H + i * SL, g * H + (i + 1) * SL)
                              for g in range(3)])
        m1t = np.ascontiguousarray(
            M1[idx].transpose(2, 1, 0).reshape(NIN, KT * M3)).astype(bfnp)
        im = {
            'xpad': xpad,
            'm1t': m1t,
            'w2xt': BF(W2x[idx].T),
            'w3xt': BF(W3x[idx].T),
            'whh0t': kmaj(Whh0[idx]),
            'weff2t': kmaj(Weff2[idx]),
            'whh1t': kmaj(Whh1[idx]),
            'w3h1t': kmaj(W3h1[idx]),
            'whh2t': kmaj(Whh2[idx]),
            'ft': BF(np.concatenate(
                [F1[:, i * SL:(i + 1) * SL].T,
                 F2[:, i * SL:(i + 1) * SL].T,
                 F3[:, i * SL:(i + 1) * SL].T], axis=1)),
            'xc1': np.ascontiguousarray(
                xc1[idx].reshape(3, SL, B).transpose(1, 0, 2).reshape(SL, 3 * B)),
            'xc2': np.ascontiguousarray(
                xc2[idx].reshape(3, SL, B).transpose(1, 0, 2).reshape(SL, 3 * B)),
            'xc3': np.ascontiguousarray(
                xc3[idx].reshape(3, SL, B).transpose(1, 0, 2).reshape(SL, 3 * B)),
            'h1t0': h1T, 'h2t0': h2T, 'h3t0': h3T,
            'h1own0': np.ascontiguousarray(A(h1).T[i * SL:(i + 1) * SL]),
            'h2own0': np.ascontiguousarray(A(h2).T[i * SL:(i + 1) * SL]),
            'h3own0': np.ascontiguousarray(A(h3).T[i * SL:(i + 1) * SL]),
        }
        in_maps.append(im)
    return in_maps, yb


def _run(x, cond, h1, h2, h3, params, t_steps=T, trace=False):
    nc = _build(t_steps)
    in_maps, yb = _prepare(x, cond, h1, h2, h3, params, t_steps)
    res = bass_utils.run_bass_kernel_spmd(
        nc, in_maps, core_ids=list(range(NCORES)), trace=trace)
    chunks = [res.results[i]['yout'] for i in range(NCORES)]
    y = np.concatenate(chunks).reshape(t_steps, NIN, B).transpose(0, 2, 1)
    y = y + yb[None, None, :]
    return np.ascontiguousarray(y, dtype=np.float32), res


def kernel(x, cond, h1, h2, h3, params):
    y, _ = _run(x, cond, h1, h2, h3, params)
    return y


# revision 13
# speedup vs baseline: 1.0450x; 1.0375x over previous
"""Trainium2 Bass kernel for nn_RNNModelWithConditioning.

Strategy: 8-way model-parallel over the hidden dimension (each core owns a
128-row slice of every gate block), full batch per core (N=256). All
weights stay SBUF-resident across the T=256 recurrence. Host-side numpy
folds the conv + input-side linears + cross-layer links into per-step
matmul-only forms:

  gx1[t] = sum_kt M1[:,kt,:] @ xpad[t+kt]          (conv+Wih0 folded)
  gx2[t] = W2x @ x0[t] + Weff2 @ h1[t] + xc2       (in1/ht0 folded)
  gx3[t] = W3x @ x0[t] + W3h1 @ h1[t] + xc3        (in2/ht1/ht2/ht0 folded)
  y[t]   = F1 @ h1[t] + F2 @ h2[t] + F3 @ h3[t] + yb

Matmul operands are bf16 (weights, x windows, gathered h); the local GRU
state h_own is kept in f32 so rounding does not compound step to step.
The loop is software-pipelined: layer 1 of step t+1 is emitted before
layers 2/3 of step t, so each AllGather's round trip hides under the
other half-step's matmuls. y partials (feature-sliced) accumulate in DRAM
and are ReduceScattered at the end; the host concatenates the 8 chunks.
"""
import numpy as np
import ml_dtypes

import concourse.bass as bass
import concourse.bacc as bacc
import concourse.tile as tile
import concourse.mybir as mybir
from concourse import bass_utils
from concourse.tile_rust import add_dep_helper

B = 256
T = 256
H = 1024
NCORES = 8
SL = 128          # hidden slice per core
NIN = 81
KT = 11           # conv time taps
M3 = 3 * SL       # 384 rows per core (r|z|n)

f32 = mybir.dt.float32
bf16 = mybir.dt.bfloat16
AF = mybir.ActivationFunctionType
bfnp = ml_dtypes.bfloat16

_nc_cache = {}


def _build(t_steps):
    if t_steps in _nc_cache:
        return _nc_cache[t_steps]
    nc = bacc.Bacc("TRN2", target_bir_lowering=False, debug=False,
                   num_devices=NCORES)
    din = {}

    def inp(name, shape, dt=bf16):
        din[name] = nc.dram_tensor(name, shape, dt, kind="ExternalInput").ap()

    inp('xpad', [t_steps + 10, NIN, B])
    inp('m1t', [NIN, KT * M3])
    inp('w2xt', [NIN, M3])
    inp('w3xt', [NIN, M3])
    for nm in ('whh0t', 'weff2t', 'whh1t', 'w3h1t', 'whh2t'):
        inp(nm, [128, 8 * M3])
    inp('ft', [128, 3 * NIN])
    for nm in ('xc1', 'xc2', 'xc3'):
        inp(nm, [128, 3 * B], f32)
    for nm in ('h1t0', 'h2t0', 'h3t0'):
        inp(nm, [128, 8 * B])
    for nm in ('h1own0', 'h2own0', 'h3own0'):
        inp(nm, [128, B], f32)
    ychunk = t_steps * NIN * B // NCORES
    yout = nc.dram_tensor('yout', [ychunk], f32, kind="ExternalOutput").ap()

    with tile.TileContext(nc) as tc:
        with tc.tile_pool(name="wpool", bufs=1) as wp, \
             tc.tile_pool(name="hpool", bufs=2) as hp, \
             tc.tile_pool(name="xwpool", bufs=14) as xp, \
             tc.tile_pool(name="scratch", bufs=1) as sp, \
             tc.tile_pool(name="ownpool", bufs=2) as op_, \
             tc.tile_pool(name="pspool", bufs=1, space="PSUM") as pp, \
             tc.tile_pool(name="drampool", bufs=2, space="DRAM") as dp:

            # ---- load constants ----
            W = {}
            for nm, shape in (('m1t', [NIN, KT * M3]), ('w2xt', [NIN, M3]),
                              ('w3xt', [NIN, M3]), ('whh0t', [128, 8 * M3]),
                              ('weff2t', [128, 8 * M3]), ('whh1t', [128, 8 * M3]),
                              ('w3h1t', [128, 8 * M3]), ('whh2t', [128, 8 * M3]),
                              ('ft', [128, 3 * NIN])):
                w_t = wp.tile(shape, bf16, tag=nm, name=nm + '_sb')
                nc.sync.dma_start(out=w_t[:], in_=din[nm][:])
                W[nm] = w_t
            XC = {}
            for nm in ('xc1', 'xc2', 'xc3'):
                c_t = wp.tile([128, 3 * B], f32, tag=nm, name=nm + '_sb')
                nc.sync.dma_start(out=c_t[:], in_=din[nm][:])
                XC[nm] = c_t

            hT = {}
            h1t_i = hp.tile([128, 8 * B], bf16, tag='h1T', name='h1T_i')
            nc.sync.dma_start(out=h1t_i[:], in_=din['h1t0'][:])
            hT[1] = h1t_i
            h23_i = hp.tile([128, 16 * B], bf16, tag='h23T', name='h23T_i')
            nc.sync.dma_start(out=h23_i[:, :8 * B], in_=din['h2t0'][:])
            nc.sync.dma_start(out=h23_i[:, 8 * B:], in_=din['h3t0'][:])
            hT[2], hT[3] = h23_i[:, :8 * B], h23_i[:, 8 * B:]
            hown = {}
            for li, nm in ((1, 'h1own0'), (2, 'h2own0'), (3, 'h3own0')):
                h_t = op_.tile([128, B], f32, tag=f'h{li}own', name=f'h{li}own_i')
                nc.sync.dma_start(out=h_t[:], in_=din[nm][:])
                hown[li] = h_t
            hbf = {}   # bf16 copies of own slices (for y matmuls / transport)
            h1gs = {}  # per-step h1 gather tiles (consumed by l23)
            h1bfs = {}  # per-step h1 bf16 slices (consumed by l23's y matmul)

            ypart = dp.tile([t_steps, NIN, B], f32, tag='ypart', name='ypart',
                            bufs=1)

            # ---- x window ring ----
            xw = {}

            def load_xw(j):
                x_t = xp.tile([NIN, B], bf16, tag='xw', name=f'xw{j}')
                nc.sync.dma_start(out=x_t[:], in_=din['xpad'][j])
                xw[j] = x_t

            for j in range(min(13, t_steps + 10)):
                load_xw(j)

            def mmgroup(dst, pairs, first=True, last=True, after=None):
                insts = []
                n = len(pairs)
                for idx, (lh, rh) in enumerate(pairs):
                    bi = nc.tensor.matmul(dst, lh, rh,
                                          start=(first and idx == 0),
                                          stop=(last and idx == n - 1))
                    insts.append(bi)
                if after is not None:
                    add_dep_helper(insts[0].ins, after[-1].ins,
                                   reason="psum bank group order")
                return insts

            def gru_gates(ps_r, ps_z, ps_xn, ps_hn, xc, li, lname):
                """Gate math; updates hown[li] (f32) and hbf[li] (bf16)."""
                tmp = sp.tile([128, B], f32, tag=f'{lname}tmp', name=f'{lname}tmp')
                nc.vector.tensor_add(tmp[:], ps_r, xc[:, :B])
                r = sp.tile([128, B], f32, tag=f'{lname}r', name=f'{lname}r')
                nc.scalar.activation(r[:], tmp[:], AF.Sigmoid)
                tmp2 = sp.tile([128, B], f32, tag=f'{lname}tmp2',
                               name=f'{lname}tmp2')
                nc.vector.tensor_add(tmp2[:], ps_z, xc[:, B:2 * B])
                z = sp.tile([128, B], f32, tag=f'{lname}z', name=f'{lname}z')
                nc.scalar.activation(z[:], tmp2[:], AF.Sigmoid)
                t1 = sp.tile([128, B], f32, tag=f'{lname}t1', name=f'{lname}t1')
                nc.vector.tensor_mul(t1[:], r[:], ps_hn)
                t2 = sp.tile([128, B], f32, tag=f'{lname}t2', name=f'{lname}t2')
                nc.vector.tensor_add(t2[:], t1[:], ps_xn)
                t3 = sp.tile([128, B], f32, tag=f'{lname}t3', name=f'{lname}t3')
                nc.vector.tensor_add(t3[:], t2[:], xc[:, 2 * B:3 * B])
                n_t = sp.tile([128, B], f32, tag=f'{lname}n', name=f'{lname}n')
                nc.scalar.activation(n_t[:], t3[:], AF.Tanh)
                d = sp.tile([128, B], f32, tag=f'{lname}d', name=f'{lname}d')
                nc.vector.tensor_sub(d[:], hown[li][:], n_t[:])
                e = sp.tile([128, B], f32, tag=f'{lname}e', name=f'{lname}e')
                nc.vector.tensor_mul(e[:], z[:], d[:])
                h_new = op_.tile([128, B], f32, tag=f'{lname}own',
                                 name=f'{lname}own')
                nc.vector.tensor_add(h_new[:], n_t[:], e[:])
                hown[li] = h_new
                h_b = op_.tile([128, B], bf16, tag=f'{lname}bf', name=f'{lname}bf')
                nc.scalar.copy(h_b[:], h_new[:])
                hbf[li] = h_b

            def l1(t):
                """Layer-1 of step t: x-side matmuls are emitted before the
                h-side (which waits on AG1(t-1)); produces h1[t] slice + AG1."""
                j = t + 12
                if j < t_steps + 10:
                    load_xw(j)
                psA = pp.tile([128, 2 * B], f32, tag='psA1', name='psA1')
                psB = pp.tile([128, 2 * B], f32, tag='psB1', name='psB1')
                m1, h1c = W['m1t'], hT[1]

                def m1_pairs(g):
                    return [(m1[:, kt * M3 + g * SL: kt * M3 + (g + 1) * SL],
                             xw[t + kt][:]) for kt in range(KT)]

                def whh_pairs(g):
                    return [(W['whh0t'][:, k * M3 + g * SL: k * M3 + (g + 1) * SL],
                             h1c[:, k * B:(k + 1) * B]) for k in range(8)]

                # early (x-only): xn, r-x, z-x
                g_xn = mmgroup(psB[:, :B], m1_pairs(2))
                g_rx = mmgroup(psA[:, :B], m1_pairs(0), last=False)
                g_zx = mmgroup(psB[:, B:2 * B], m1_pairs(1), last=False,
                               after=g_xn)
                # late (need h1 gather): r-h, z-h, hn
                g_rh = mmgroup(psA[:, :B], whh_pairs(0), first=False)
                g_zh = mmgroup(psB[:, B:2 * B], whh_pairs(1), first=False)
                mmgroup(psA[:, B:2 * B], whh_pairs(2), after=g_rh)

                gru_gates(psA[:, :B], psB[:, B:2 * B], psB[:, :B],
                          psA[:, B:2 * B], XC['xc1'], 1, 'L1')

                agin1 = dp.tile([128, B], bf16, tag='agin1', name='agin1', bufs=2)
                nc.sync.dma_start(out=agin1[:], in_=hbf[1][:])
                agout1 = dp.tile([NCORES, 128, B], bf16, tag='agout1',
                                 name='agout1', addr_space="Shared", bufs=2)
                nc.gpsimd.collective_compute(
                    "AllGather", mybir.AluOpType.bypass,
                    replica_groups=[list(range(NCORES))],
                    ins=[agin1[:].opt()], outs=[agout1[:].opt()])
                h1g = hp.tile([128, 8 * B], bf16, tag='h1T', name='h1T')
                nc.gpsimd.dma_start(out=h1g[:],
                                    in_=agout1[:].rearrange("k p b -> p k b"))
                hT[1] = h1g
                h1gs[t] = h1g
                h1bfs[t] = hbf[1]

            L23 = (
                (2, 'w2xt', 'whh1t', 'weff2t', 'xc2', 'L2'),
                (3, 'w3xt', 'whh2t', 'w3h1t', 'xc3', 'L3'))

            def l23_head(t, st):
                """Early (AG-independent) matmuls of layers 2/3 of step t:
                w2x + whh parts of the r and z gates."""
                for li, wx, whh, weff, xc, lname in L23:
                    hc = hT[li]
                    psA = pp.tile([128, 2 * B], f32, tag='psA23',
                                  name=f'psA{li}', bufs=2)
                    psB = pp.tile([128, 2 * B], f32, tag='psB23',
                                  name=f'psB{li}', bufs=2)
                    st[li] = (psA, psB)
                    for g, dst in ((0, psA[:, :B]), (1, psB[:, :B])):
                        pairs = [(W[wx][:, g * SL:(g + 1) * SL], xw[t + 5][:])]
                        pairs += [(W[whh][:, k * M3 + g * SL: k * M3 + (g + 1) * SL],
                                   hc[:, k * B:(k + 1) * B]) for k in range(8)]
                        mmgroup(dst, pairs, last=False)

            def l23_tail(t, st):
                """Late matmuls (weff @ h1[t], hn, xn) + gates + AG23 + y."""
                h1c = h1gs.pop(t)
                h1b = h1bfs.pop(t)
                for li, wx, whh, weff, xc, lname in L23:
                    hc = hT[li]
                    psA, psB = st[li]

                    def weff_pairs(g):
                        return [(W[weff][:, k * M3 + g * SL: k * M3 + (g + 1) * SL],
                                 h1c[:, k * B:(k + 1) * B]) for k in range(8)]

                    g_rl = mmgroup(psA[:, :B], weff_pairs(0), first=False)
                    g_zl = mmgroup(psB[:, :B], weff_pairs(1), first=False)
                    mmgroup(psA[:, B:2 * B],
                            [(W[whh][:, k * M3 + 2 * SL: k * M3 + 3 * SL],
                              hc[:, k * B:(k + 1) * B]) for k in range(8)],
                            after=g_rl)
                    mmgroup(psB[:, B:2 * B],
                            [(W[wx][:, 2 * SL:3 * SL], xw[t + 5][:])]
                            + weff_pairs(2), after=g_zl)
                    gru_gates(psA[:, :B], psB[:, :B], psB[:, B:2 * B],
                              psA[:, B:2 * B], XC[xc], li, lname)

                # AllGather h2 & h3 (skip after last step)
                if t + 1 < t_steps:
                    agin23 = dp.tile([2, 128, B], bf16, tag='agin23',
                                     name='agin23', bufs=2)
                    nc.sync.dma_start(out=agin23[0], in_=hbf[2][:])
                    nc.sync.dma_start(out=agin23[1], in_=hbf[3][:])
                    agout23 = dp.tile([NCORES, 2, 128, B], bf16, tag='agout23',
                                      name='agout23', addr_space="Shared", bufs=2)
                    nc.gpsimd.collective_compute(
                        "AllGather", mybir.AluOpType.bypass,
                        replica_groups=[list(range(NCORES))],
                        ins=[agin23[:].opt()], outs=[agout23[:].opt()])
                    h23g = hp.tile([128, 16 * B], bf16, tag='h23T', name='h23T')
                    nc.gpsimd.dma_start(
                        out=h23g[:, :8 * B],
                        in_=agout23[:, 0].rearrange("k p b -> p k b"))
                    nc.gpsimd.dma_start(
                        out=h23g[:, 8 * B:],
                        in_=agout23[:, 1].rearrange("k p b -> p k b"))
                    hT[2], hT[3] = h23g[:, :8 * B], h23g[:, 8 * B:]

                # y partials from own slices
                ps_y = pp.tile([NIN, B], f32, tag='yps', name='yps')
                nc.tensor.matmul(ps_y[:], W['ft'][:, 0:NIN], h1b[:],
                                 start=True, stop=False)
                nc.tensor.matmul(ps_y[:], W['ft'][:, NIN:2 * NIN], hbf[2][:],
                                 start=False, stop=False)
                nc.tensor.matmul(ps_y[:], W['ft'][:, 2 * NIN:3 * NIN], hbf[3][:],
                                 start=False, stop=True)
                ysb = sp.tile([NIN, B], f32, tag='ysb', name='ysb')
                nc.vector.tensor_copy(ysb[:], ps_y[:])
                nc.sync.dma_start(out=ypart[t], in_=ysb[:])
                xw.pop(t - 1, None)

            # ---- software-pipelined loop ----
            l1(0)
            for t in range(t_steps):
                st = {}
                if t + 1 < t_steps:
                    l1(t + 1)
                l23_head(t, st)
                l23_tail(t, st)

            # ---- final ReduceScatter of y partials ----
            yred = dp.tile([t_steps * NIN * B // NCORES], f32, tag='yred',
                           name='yred', bufs=1)
            nc.gpsimd.collective_compute(
                "ReduceScatter", mybir.AluOpType.add,
                replica_groups=[list(range(NCORES))],
                ins=[ypart[:].opt()], outs=[yred[:].opt()])
            nc.sync.dma_start(out=yout[:], in_=yred[:])

    nc.compile()
    _nc_cache[t_steps] = nc
    return nc


def _prepare(x, cond, h1, h2, h3, params, t_steps):
    """Host-side folding. Returns (in_maps, yb)."""
    p = params
    fp = np.float32

    def A(v):
        return np.ascontiguousarray(np.asarray(v), dtype=fp)

    def BF(v):
        return np.ascontiguousarray(np.asarray(v, dtype=fp)).astype(bfnp)

    Wih0, Whh0 = A(p['Wih0']), A(p['Whh0'])
    Wih1, Whh1 = A(p['Wih1']), A(p['Whh1'])
    Wih2, Whh2 = A(p['Wih2']), A(p['Whh2'])
    conv_w = A(p['conv_w'])[:, 0]          # [64, 21, 11]
    conv_b = A(p['conv_b'])
    cond_np = A(cond)

    cond1 = cond_np @ A(p['cond0_w']).T + A(p['cond0_b'])
    cond2 = cond_np @ A(p['cond1_w']).T + A(p['cond1_b'])
    cond3 = cond_np @ A(p['cond2_w']).T + A(p['cond2_b'])

    # --- M1 fold: conv + Wih0 ---
    Wr = Wih0[:, :1984].reshape(3 * H, 64, 31)
    M1 = np.zeros((3 * H, KT, NIN), np.float32)
    for kf in range(21):
        tmp = np.einsum('rcf,ck->rfk', Wr, conv_w[:, kf, :], optimize=True)
        fins = 2 * np.arange(31) + kf
        M1[:, :, fins] += tmp.transpose(0, 2, 1)
    c1 = Wih0[:, :1984] @ np.repeat(conv_b, 31)
    xc1 = Wih0[:, 1984:] @ cond1.T + c1[:, None]

    W2x = Wih1[:, :H] @ A(p['in1_w'])
    Weff2 = Wih1[:, :H] @ A(p['ht0_w'])
    xc2 = (Wih1[:, :H] @ (A(p['in1_b']) + A(p['ht0_b'])))[:, None] \
        + Wih1[:, H:] @ cond2.T

    W3x = Wih2[:, :H] @ A(p['in2_w'])
    W3h1 = Wih2[:, :H] @ (A(p['ht1_w']) + A(p['ht2_w']) @ A(p['ht0_w']))
    xc3 = (Wih2[:, :H] @ (A(p['in2_b']) + A(p['ht1_b']) + A(p['ht2_b'])
                          + A(p['ht2_w']) @ A(p['ht0_b'])))[:, None] \
        + Wih2[:, H:] @ cond3.T

    F1 = A(p['final_w']) @ A(p['out0_w'])
    F2 = A(p['final_w']) @ A(p['out1_w'])
    F3 = A(p['final_w']) @ A(p['out2_w'])
    yb = A(p['final_w']) @ (A(p['out0_b']) + A(p['out1_b'])
                            + A(p['out2_b'])) + A(p['final_b'])

    # --- x: [B,1,81,T] -> padded [T+10, 81, B] ---
    xs = A(x)[:, 0, :, :t_steps]                      # [B, 81, t]
    xpad = np.zeros((t_steps + 10, NIN, B), np.float32)
    xpad[5:5 + t_steps] = xs.transpose(2, 1, 0)
    xpad = xpad.astype(bfnp)

    h1T = A(h1).T.reshape(8, 128, B).transpose(1, 0, 2).reshape(128, 8 * B)
    h2T = A(h2).T.reshape(8, 128, B).transpose(1, 0, 2).reshape(128, 8 * B)
    h3T = A(h3).T.reshape(8, 128, B).transpose(1, 0, 2).reshape(128, 8 * B)
    h1T, h2T, h3T = h1T.astype(bfnp), h2T.astype(bfnp), h3T.astype(bfnp)

    def kmaj(w):          # [384, 1024] -> lhsT sbuf layout [128, 8*384]
        return np.ascontiguousarray(
            w.T.reshape(8, 128, M3).transpose(1, 0, 2).reshape(128, 8 * M3)
        ).astype(bfnp)

    in_maps = []
    for i in range(NCORES):
        idx = np.concatenate([np.arange(g * H + i * SL, g * H + (i + 1) * SL)
                              for g in range(3)])
        m1t = np.ascontiguousarray(
            M1[idx].transpose(2, 1, 0).reshape(NIN, KT * M3)).astype(bfnp)
        im = {
            'xpad': xpad,
            'm1t': m1t,
            'w2xt': BF(W2x[idx].T),
            'w3xt': BF(W3x[idx].T),
            'whh0t': kmaj(Whh0[idx]),
            'weff2t': kmaj(Weff2[idx]),
            'whh1t': kmaj(Whh1[idx]),
            'w3h1t': kmaj(W3h1[idx]),
            'whh2t': kmaj(Whh2[idx]),
            'ft': BF(np.concatenate(
                [F1[:, i * SL:(i + 1) * SL].T,
                 F2[:, i * SL:(i + 1) * SL].T,
                 F3[:, i * SL:(i + 1) * SL].T], axis=1)),
            'xc1': np.ascontiguousarray(
                xc1[idx].reshape(3, SL, B).transpose(1, 0, 2).reshape(SL, 3 * B)),
            'xc2': np.ascontiguousarray(
                xc2[idx].reshape(3, SL, B).transpose(1, 0, 2).reshape(SL, 3 * B)),
            'xc3': np.ascontiguousarray(
                xc3[idx].reshape(3, SL, B).transpose(1, 0, 2).reshape(SL, 3 * B)),
            'h1t0': h1T, 'h2t0': h2T, 'h3t0': h3T,
            'h1own0': np.ascontiguousarray(A(h1).T[i * SL:(i + 1) * SL]),
            'h2own0': np.ascontiguousarray(A(h2).T[i * SL:(i + 1) * SL]),
            'h3own0': np.ascontiguousarray(A(h3).T[i * SL:(i + 1) * SL]),
        }
        in_maps.append(im)
    return in_maps, yb


def _run(x, cond, h1, h2, h3, params, t_steps=T, trace=False):
    nc = _build(t_steps)
    in_maps, yb = _prepare(x, cond, h1, h2, h3, params, t_steps)
    res = bass_utils.run_bass_kernel_spmd(
        nc, in_maps, core_ids=list(range(NCORES)), trace=trace)
    chunks = [res.results[i]['yout'] for i in range(NCORES)]
    y = np.concatenate(chunks).reshape(t_steps, NIN, B).transpose(0, 2, 1)
    y = y + yb[None, None, :]
    return np.ascontiguousarray(y, dtype=np.float32), res


def kernel(x, cond, h1, h2, h3, params):
    y, _ = _run(x, cond, h1, h2, h3, params)
    return y


# revision 14
# speedup vs baseline: 1.0838x; 1.0371x over previous
"""Trainium2 Bass kernel for nn_RNNModelWithConditioning.

Strategy: 8-way model-parallel over the hidden dimension (each core owns a
128-row slice of every gate block), full batch per core (N=256). All
weights stay SBUF-resident across the T=256 recurrence. Host-side numpy
folds the conv + input-side linears + cross-layer links into per-step
matmul-only forms:

  gx1[t] = sum_kt M1[:,kt,:] @ xpad[t+kt]          (conv+Wih0 folded)
  gx2[t] = W2x @ x0[t] + Weff2 @ h1[t] + xc2       (in1/ht0 folded)
  gx3[t] = W3x @ x0[t] + W3h1 @ h1[t] + xc3        (in2/ht1/ht2/ht0 folded)
  y[t]   = F1 @ h1[t] + F2 @ h2[t] + F3 @ h3[t] + yb

Matmul operands are bf16 (weights, x windows, gathered h); the local GRU
state h_own is kept in f32 so rounding does not compound step to step.
The loop is software-pipelined: layer 1 of step t+1 is emitted before
layers 2/3 of step t, so each AllGather's round trip hides under the
other half-step's matmuls. y partials (feature-sliced) accumulate in DRAM
and are ReduceScattered at the end; the host concatenates the 8 chunks.
"""
import numpy as np
import ml_dtypes

import concourse.bass as bass
import concourse.bacc as bacc
import concourse.tile as tile
import concourse.mybir as mybir
from concourse import bass_utils
from concourse.tile_rust import add_dep_helper

B = 256
T = 256
H = 1024
NCORES = 8
SL = 128          # hidden slice per core
NIN = 81
KT = 11           # conv time taps
M3 = 3 * SL       # 384 rows per core (r|z|n)

f32 = mybir.dt.float32
bf16 = mybir.dt.bfloat16
AF = mybir.ActivationFunctionType
bfnp = ml_dtypes.bfloat16

_nc_cache = {}


def _build(t_steps):
    if t_steps in _nc_cache:
        return _nc_cache[t_steps]
    nc = bacc.Bacc("TRN2", target_bir_lowering=False, debug=False,
                   num_devices=NCORES)
    din = {}

    def inp(name, shape, dt=bf16):
        din[name] = nc.dram_tensor(name, shape, dt, kind="ExternalInput").ap()

    inp('xpad', [t_steps + 10, NIN, B])
    inp('m1t', [NIN, KT * M3])
    inp('w2xt', [NIN, M3])
    inp('w3xt', [NIN, M3])
    for nm in ('whh0t', 'weff2t', 'whh1t', 'w3h1t', 'whh2t'):
        inp(nm, [128, 8 * M3])
    inp('ft', [128, 3 * NIN])
    for nm in ('xc1', 'xc2', 'xc3'):
        inp(nm, [128, 3 * B])
    inp('ident', [128, 128])
    for nm in ('h1t0', 'h2t0', 'h3t0'):
        inp(nm, [128, 8 * B])
    for nm in ('h1own0', 'h2own0', 'h3own0'):
        inp(nm, [128, B], f32)
    ychunk = t_steps * NIN * B // NCORES
    yout = nc.dram_tensor('yout', [ychunk], f32, kind="ExternalOutput").ap()

    with tile.TileContext(nc) as tc:
        with tc.tile_pool(name="wpool", bufs=1) as wp, \
             tc.tile_pool(name="hpool", bufs=2) as hp, \
             tc.tile_pool(name="xwpool", bufs=14) as xp, \
             tc.tile_pool(name="scratch", bufs=1) as sp, \
             tc.tile_pool(name="ownpool", bufs=2) as op_, \
             tc.tile_pool(name="pspool", bufs=1, space="PSUM") as pp, \
             tc.tile_pool(name="drampool", bufs=2, space="DRAM") as dp:

            # ---- load constants ----
            W = {}
            for nm, shape in (('m1t', [NIN, KT * M3]), ('w2xt', [NIN, M3]),
                              ('w3xt', [NIN, M3]), ('whh0t', [128, 8 * M3]),
                              ('weff2t', [128, 8 * M3]), ('whh1t', [128, 8 * M3]),
                              ('w3h1t', [128, 8 * M3]), ('whh2t', [128, 8 * M3]),
                              ('ft', [128, 3 * NIN])):
                w_t = wp.tile(shape, bf16, tag=nm, name=nm + '_sb')
                nc.sync.dma_start(out=w_t[:], in_=din[nm][:])
                W[nm] = w_t
            XC = {}
            for nm in ('xc1', 'xc2', 'xc3'):
                c_t = wp.tile([128, 3 * B], bf16, tag=nm, name=nm + '_sb')
                nc.sync.dma_start(out=c_t[:], in_=din[nm][:])
                XC[nm] = c_t
            ident = wp.tile([128, 128], bf16, tag='ident', name='ident_sb')
            nc.sync.dma_start(out=ident[:], in_=din['ident'][:])

            hT = {}
            h1t_i = hp.tile([128, 8 * B], bf16, tag='h1T', name='h1T_i')
            nc.sync.dma_start(out=h1t_i[:], in_=din['h1t0'][:])
            hT[1] = h1t_i
            h23_i = hp.tile([128, 16 * B], bf16, tag='h23T', name='h23T_i')
            nc.sync.dma_start(out=h23_i[:, :8 * B], in_=din['h2t0'][:])
            nc.sync.dma_start(out=h23_i[:, 8 * B:], in_=din['h3t0'][:])
            hT[2], hT[3] = h23_i[:, :8 * B], h23_i[:, 8 * B:]
            hown = {}
            for li, nm in ((1, 'h1own0'), (2, 'h2own0'), (3, 'h3own0')):
                h_t = op_.tile([128, B], f32, tag=f'h{li}own', name=f'h{li}own_i')
                nc.sync.dma_start(out=h_t[:], in_=din[nm][:])
                hown[li] = h_t
            hbf = {}   # bf16 copies of own slices (for y matmuls / transport)
            h1gs = {}  # per-step h1 gather tiles (consumed by l23)
            h1bfs = {}  # per-step h1 bf16 slices (consumed by l23's y matmul)

            ypart = dp.tile([t_steps, NIN, B], f32, tag='ypart', name='ypart',
                            bufs=1)

            # ---- x window ring ----
            xw = {}

            def load_xw(j):
                x_t = xp.tile([NIN, B], bf16, tag='xw', name=f'xw{j}')
                nc.sync.dma_start(out=x_t[:], in_=din['xpad'][j])
                xw[j] = x_t

            for j in range(min(13, t_steps + 10)):
                load_xw(j)

            def mmgroup(dst, pairs, first=True, last=True, after=None):
                insts = []
                n = len(pairs)
                for idx, (lh, rh) in enumerate(pairs):
                    bi = nc.tensor.matmul(dst, lh, rh,
                                          start=(first and idx == 0),
                                          stop=(last and idx == n - 1))
                    insts.append(bi)
                if after is not None:
                    add_dep_helper(insts[0].ins, after[-1].ins,
                                   reason="psum bank group order")
                return insts

            def gru_gates(ps_r, ps_z, ps_xn, ps_hn, li, lname):
                """Gate math (xc already accumulated into psums via identity
                matmuls). Critical chain: sigmoid -> t1 -> t2 -> tanh -> p ->
                h_bf; z/w/u branch runs in parallel."""
                r = sp.tile([128, B], f32, tag=f'{lname}r', name=f'{lname}r')
                nc.scalar.activation(r[:], ps_r, AF.Sigmoid)
                z = sp.tile([128, B], f32, tag=f'{lname}z', name=f'{lname}z')
                nc.scalar.activation(z[:], ps_z, AF.Sigmoid)
                w = sp.tile([128, B], f32, tag=f'{lname}w', name=f'{lname}w')
                nc.vector.tensor_scalar(w[:], z[:], -1.0, 1.0,
                                        mybir.AluOpType.mult,
                                        mybir.AluOpType.add)
                u = sp.tile([128, B], f32, tag=f'{lname}u', name=f'{lname}u')
                nc.vector.tensor_mul(u[:], z[:], hown[li][:])
                t1 = sp.tile([128, B], f32, tag=f'{lname}t1', name=f'{lname}t1')
                nc.vector.tensor_mul(t1[:], r[:], ps_hn)
                t2 = sp.tile([128, B], f32, tag=f'{lname}t2', name=f'{lname}t2')
                nc.vector.tensor_add(t2[:], t1[:], ps_xn)
                n_t = sp.tile([128, B], f32, tag=f'{lname}n', name=f'{lname}n')
                nc.scalar.activation(n_t[:], t2[:], AF.Tanh)
                p = sp.tile([128, B], f32, tag=f'{lname}p', name=f'{lname}p')
                nc.vector.tensor_mul(p[:], n_t[:], w[:])
                h_b = op_.tile([128, B], bf16, tag=f'{lname}bf', name=f'{lname}bf')
                nc.vector.tensor_add(h_b[:], p[:], u[:])
                hbf[li] = h_b
                h_new = op_.tile([128, B], f32, tag=f'{lname}own',
                                 name=f'{lname}own')
                nc.vector.tensor_add(h_new[:], p[:], u[:])
                hown[li] = h_new

            def l1(t):
                """Layer-1 of step t: x-side matmuls are emitted before the
                h-side (which waits on AG1(t-1)); produces h1[t] slice + AG1."""
                j = t + 12
                if j < t_steps + 10:
                    load_xw(j)
                psA = pp.tile([128, 2 * B], f32, tag='psA1', name='psA1')
                psB = pp.tile([128, 2 * B], f32, tag='psB1', name='psB1')
                m1, h1c = W['m1t'], hT[1]

                def m1_pairs(g):
                    return [(m1[:, kt * M3 + g * SL: kt * M3 + (g + 1) * SL],
                             xw[t + kt][:]) for kt in range(KT)]

                def whh_pairs(g):
                    return [(W['whh0t'][:, k * M3 + g * SL: k * M3 + (g + 1) * SL],
                             h1c[:, k * B:(k + 1) * B]) for k in range(8)]

                # early (x-only): xn, r-x, z-x (ident@xc folds the constants)
                xc = XC['xc1']
                g_xn = mmgroup(psB[:, :B],
                               [(ident[:], xc[:, 2 * B:3 * B])] + m1_pairs(2))
                g_rx = mmgroup(psA[:, :B],
                               [(ident[:], xc[:, 0:B])] + m1_pairs(0),
                               last=False)
                g_zx = mmgroup(psB[:, B:2 * B],
                               [(ident[:], xc[:, B:2 * B])] + m1_pairs(1),
                               last=False, after=g_xn)
                # late (need h1 gather): r-h, z-h, hn
                g_rh = mmgroup(psA[:, :B], whh_pairs(0), first=False)
                g_zh = mmgroup(psB[:, B:2 * B], whh_pairs(1), first=False)
                mmgroup(psA[:, B:2 * B], whh_pairs(2), after=g_rh)

                gru_gates(psA[:, :B], psB[:, B:2 * B], psB[:, :B],
                          psA[:, B:2 * B], 1, 'L1')

                agin1 = dp.tile([128, B], bf16, tag='agin1', name='agin1', bufs=2)
                nc.sync.dma_start(out=agin1[:], in_=hbf[1][:])
                agout1 = dp.tile([NCORES, 128, B], bf16, tag='agout1',
                                 name='agout1', addr_space="Shared", bufs=2)
                nc.gpsimd.collective_compute(
                    "AllGather", mybir.AluOpType.bypass,
                    replica_groups=[list(range(NCORES))],
                    ins=[agin1[:].opt()], outs=[agout1[:].opt()])
                h1g = hp.tile([128, 8 * B], bf16, tag='h1T', name='h1T')
                nc.gpsimd.dma_start(out=h1g[:],
                                    in_=agout1[:].rearrange("k p b -> p k b"))
                hT[1] = h1g
                h1gs[t] = h1g
                h1bfs[t] = hbf[1]

            L23 = (
                (2, 'w2xt', 'whh1t', 'weff2t', 'xc2', 'L2'),
                (3, 'w3xt', 'whh2t', 'w3h1t', 'xc3', 'L3'))

            def l23_head(t, st):
                """Early (AG-independent) matmuls of layers 2/3 of step t:
                w2x + whh parts of the r and z gates."""
                for li, wx, whh, weff, xc, lname in L23:
                    hc = hT[li]
                    psA = pp.tile([128, 2 * B], f32, tag='psA23',
                                  name=f'psA{li}', bufs=2)
                    psB = pp.tile([128, 2 * B], f32, tag='psB23',
                                  name=f'psB{li}', bufs=2)
                    st[li] = (psA, psB)
                    for g, dst in ((0, psA[:, :B]), (1, psB[:, :B])):
                        pairs = [(ident[:], XC[xc][:, g * B:(g + 1) * B]),
                                 (W[wx][:, g * SL:(g + 1) * SL], xw[t + 5][:])]
                        pairs += [(W[whh][:, k * M3 + g * SL: k * M3 + (g + 1) * SL],
                                   hc[:, k * B:(k + 1) * B]) for k in range(8)]
                        mmgroup(dst, pairs, last=False)

            def l23_tail(t, st):
                """Late matmuls (weff @ h1[t], hn, xn) + gates + AG23 + y."""
                h1c = h1gs.pop(t)
                h1b = h1bfs.pop(t)
                for li, wx, whh, weff, xc, lname in L23:
                    hc = hT[li]
                    psA, psB = st[li]

                    def weff_pairs(g):
                        return [(W[weff][:, k * M3 + g * SL: k * M3 + (g + 1) * SL],
                                 h1c[:, k * B:(k + 1) * B]) for k in range(8)]

                    g_rl = mmgroup(psA[:, :B], weff_pairs(0), first=False)
                    g_zl = mmgroup(psB[:, :B], weff_pairs(1), first=False)
                    mmgroup(psA[:, B:2 * B],
                            [(W[whh][:, k * M3 + 2 * SL: k * M3 + 3 * SL],
                              hc[:, k * B:(k + 1) * B]) for k in range(8)],
                            after=g_rl)
                    mmgroup(psB[:, B:2 * B],
                            [(ident[:], XC[xc][:, 2 * B:3 * B]),
                             (W[wx][:, 2 * SL:3 * SL], xw[t + 5][:])]
                            + weff_pairs(2), after=g_zl)
                    gru_gates(psA[:, :B], psB[:, :B], psB[:, B:2 * B],
                              psA[:, B:2 * B], li, lname)

                # AllGather h2 & h3 (skip after last step)
                if t + 1 < t_steps:
                    agin23 = dp.tile([2, 128, B], bf16, tag='agin23',
                                     name='agin23', bufs=2)
                    nc.sync.dma_start(out=agin23[0], in_=hbf[2][:])
                    nc.sync.dma_start(out=agin23[1], in_=hbf[3][:])
                    agout23 = dp.tile([NCORES, 2, 128, B], bf16, tag='agout23',
                                      name='agout23', addr_space="Shared", bufs=2)
                    nc.gpsimd.collective_compute(
                        "AllGather", mybir.AluOpType.bypass,
                        replica_groups=[list(range(NCORES))],
                        ins=[agin23[:].opt()], outs=[agout23[:].opt()])
                    h23g = hp.tile([128, 16 * B], bf16, tag='h23T', name='h23T')
                    nc.gpsimd.dma_start(
                        out=h23g[:, :8 * B],
                        in_=agout23[:, 0].rearrange("k p b -> p k b"))
                    nc.gpsimd.dma_start(
                        out=h23g[:, 8 * B:],
                        in_=agout23[:, 1].rearrange("k p b -> p k b"))
                    hT[2], hT[3] = h23g[:, :8 * B], h23g[:, 8 * B:]

                # y partials from own slices
                ps_y = pp.tile([NIN, B], f32, tag='yps', name='yps')
                nc.tensor.matmul(ps_y[:], W['ft'][:, 0:NIN], h1b[:],
                                 start=True, stop=False)
                nc.tensor.matmul(ps_y[:], W['ft'][:, NIN:2 * NIN], hbf[2][:],
                                 start=False, stop=False)
                nc.tensor.matmul(ps_y[:], W['ft'][:, 2 * NIN:3 * NIN], hbf[3][:],
                                 start=False, stop=True)
                ysb = sp.tile([NIN, B], f32, tag='ysb', name='ysb')
                nc.vector.tensor_copy(ysb[:], ps_y[:])
                nc.sync.dma_start(out=ypart[t], in_=ysb[:])
                xw.pop(t - 1, None)

            # ---- software-pipelined loop ----
            l1(0)
            for t in range(t_steps):
                st = {}
                if t + 1 < t_steps:
                    l1(t + 1)
                l23_head(t, st)
                l23_tail(t, st)

            # ---- final ReduceScatter of y partials ----
            yred = dp.tile([t_steps * NIN * B // NCORES], f32, tag='yred',
                           name='yred', bufs=1)
            nc.gpsimd.collective_compute(
                "ReduceScatter", mybir.AluOpType.add,
                replica_groups=[list(range(NCORES))],
                ins=[ypart[:].opt()], outs=[yred[:].opt()])
            nc.sync.dma_start(out=yout[:], in_=yred[:])

    nc.compile()
    _nc_cache[t_steps] = nc
    return nc


def _prepare(x, cond, h1, h2, h3, params, t_steps):
    """Host-side folding. Returns (in_maps, yb)."""
    p = params
    fp = np.float32

    def A(v):
        return np.ascontiguousarray(np.asarray(v), dtype=fp)

    def BF(v):
        return np.ascontiguousarray(np.asarray(v, dtype=fp)).astype(bfnp)

    Wih0, Whh0 = A(p['Wih0']), A(p['Whh0'])
    Wih1, Whh1 = A(p['Wih1']), A(p['Whh1'])
    Wih2, Whh2 = A(p['Wih2']), A(p['Whh2'])
    conv_w = A(p['conv_w'])[:, 0]          # [64, 21, 11]
    conv_b = A(p['conv_b'])
    cond_np = A(cond)

    cond1 = cond_np @ A(p['cond0_w']).T + A(p['cond0_b'])
    cond2 = cond_np @ A(p['cond1_w']).T + A(p['cond1_b'])
    cond3 = cond_np @ A(p['cond2_w']).T + A(p['cond2_b'])

    # --- M1 fold: conv + Wih0 ---
    Wr = Wih0[:, :1984].reshape(3 * H, 64, 31)
    M1 = np.zeros((3 * H, KT, NIN), np.float32)
    for kf in range(21):
        tmp = np.einsum('rcf,ck->rfk', Wr, conv_w[:, kf, :], optimize=True)
        fins = 2 * np.arange(31) + kf
        M1[:, :, fins] += tmp.transpose(0, 2, 1)
    c1 = Wih0[:, :1984] @ np.repeat(conv_b, 31)
    xc1 = Wih0[:, 1984:] @ cond1.T + c1[:, None]

    W2x = Wih1[:, :H] @ A(p['in1_w'])
    Weff2 = Wih1[:, :H] @ A(p['ht0_w'])
    xc2 = (Wih1[:, :H] @ (A(p['in1_b']) + A(p['ht0_b'])))[:, None] \
        + Wih1[:, H:] @ cond2.T

    W3x = Wih2[:, :H] @ A(p['in2_w'])
    W3h1 = Wih2[:, :H] @ (A(p['ht1_w']) + A(p['ht2_w']) @ A(p['ht0_w']))
    xc3 = (Wih2[:, :H] @ (A(p['in2_b']) + A(p['ht1_b']) + A(p['ht2_b'])
                          + A(p['ht2_w']) @ A(p['ht0_b'])))[:, None] \
        + Wih2[:, H:] @ cond3.T

    F1 = A(p['final_w']) @ A(p['out0_w'])
    F2 = A(p['final_w']) @ A(p['out1_w'])
    F3 = A(p['final_w']) @ A(p['out2_w'])
    yb = A(p['final_w']) @ (A(p['out0_b']) + A(p['out1_b'])
                            + A(p['out2_b'])) + A(p['final_b'])

    # --- x: [B,1,81,T] -> padded [T+10, 81, B] ---
    xs = A(x)[:, 0, :, :t_steps]                      # [B, 81, t]
    xpad = np.zeros((t_steps + 10, NIN, B), np.float32)
    xpad[5:5 + t_steps] = xs.transpose(2, 1, 0)
    xpad = xpad.astype(bfnp)

    h1T = A(h1).T.reshape(8, 128, B).transpose(1, 0, 2).reshape(128, 8 * B)
    h2T = A(h2).T.reshape(8, 128, B).transpose(1, 0, 2).reshape(128, 8 * B)
    h3T = A(h3).T.reshape(8, 128, B).transpose(1, 0, 2).reshape(128, 8 * B)
    h1T, h2T, h3T = h1T.astype(bfnp), h2T.astype(bfnp), h3T.astype(bfnp)

    def kmaj(w):          # [384, 1024] -> lhsT sbuf layout [128, 8*384]
        return np.ascontiguousarray(
            w.T.reshape(8, 128, M3).transpose(1, 0, 2).reshape(128, 8 * M3)
        ).astype(bfnp)

    in_maps = []
    for i in range(NCORES):
        idx = np.concatenate([np.arange(g * H + i * SL, g * H + (i + 1) * SL)
                              for g in range(3)])
        m1t = np.ascontiguousarray(
            M1[idx].transpose(2, 1, 0).reshape(NIN, KT * M3)).astype(bfnp)
        im = {
            'xpad': xpad,
            'm1t': m1t,
            'w2xt': BF(W2x[idx].T),
            'w3xt': BF(W3x[idx].T),
            'whh0t': kmaj(Whh0[idx]),
            'weff2t': kmaj(Weff2[idx]),
            'whh1t': kmaj(Whh1[idx]),
            'w3h1t': kmaj(W3h1[idx]),
            'whh2t': kmaj(Whh2[idx]),
            'ft': BF(np.concatenate(
                [F1[:, i * SL:(i + 1) * SL].T,
                 F2[:, i * SL:(i + 1) * SL].T,
                 F3[:, i * SL:(i + 1) * SL].T], axis=1)),
            'xc1': np.ascontiguousarray(
                xc1[idx].reshape(3, SL, B).transpose(1, 0, 2)
                .reshape(SL, 3 * B)).astype(bfnp),
            'xc2': np.ascontiguousarray(
                xc2[idx].reshape(3, SL, B).transpose(1, 0, 2)
                .reshape(SL, 3 * B)).astype(bfnp),
            'xc3': np.ascontiguousarray(
                xc3[idx].reshape(3, SL, B).transpose(1, 0, 2)
                .reshape(SL, 3 * B)).astype(bfnp),
            'ident': np.eye(128, dtype=np.float32).astype(bfnp),
            'h1t0': h1T, 'h2t0': h2T, 'h3t0': h3T,
            'h1own0': np.ascontiguousarray(A(h1).T[i * SL:(i + 1) * SL]),
            'h2own0': np.ascontiguousarray(A(h2).T[i * SL:(i + 1) * SL]),
            'h3own0': np.ascontiguousarray(A(h3).T[i * SL:(i + 1) * SL]),
        }
        in_maps.append(im)
    return in_maps, yb


def _run(x, cond, h1, h2, h3, params, t_steps=T, trace=False):
    nc = _build(t_steps)
    in_maps, yb = _prepare(x, cond, h1, h2, h3, params, t_steps)
    res = bass_utils.run_bass_kernel_spmd(
        nc, in_maps, core_ids=list(range(NCORES)), trace=trace)
    chunks = [res.results[i]['yout'] for i in range(NCORES)]
    y = np.concatenate(chunks).reshape(t_steps, NIN, B).transpose(0, 2, 1)
    y = y + yb[None, None, :]
    return np.ascontiguousarray(y, dtype=np.float32), res


def kernel(x, cond, h1, h2, h3, params):
    y, _ = _run(x, cond, h1, h2, h3, params)
    return y


# revision 16
# speedup vs baseline: 1.1028x; 1.0176x over previous
"""Trainium2 Bass kernel for nn_RNNModelWithConditioning.

Strategy: 8-way model-parallel over the hidden dimension (each core owns a
128-row slice of every gate block), full batch per core (N=256). All
weights stay SBUF-resident across the T=256 recurrence. Host-side numpy
folds the conv + input-side linears + cross-layer links into per-step
matmul-only forms:

  gx1[t] = sum_kt M1[:,kt,:] @ xpad[t+kt]          (conv+Wih0 folded)
  gx2[t] = W2x @ x0[t] + Weff2 @ h1[t] + xc2       (in1/ht0 folded)
  gx3[t] = W3x @ x0[t] + W3h1 @ h1[t] + xc3        (in2/ht1/ht2/ht0 folded)
  y[t]   = F1 @ h1[t] + F2 @ h2[t] + F3 @ h3[t] + yb

Matmul operands are bf16 (weights, x windows, gathered h); the local GRU
state h_own is kept in f32 so rounding does not compound step to step.
The loop is software-pipelined: layer 1 of step t+1 is emitted before
layers 2/3 of step t, so each AllGather's round trip hides under the
other half-step's matmuls. y partials (feature-sliced) accumulate in DRAM
and are ReduceScattered at the end; the host concatenates the 8 chunks.
"""
import numpy as np
import ml_dtypes

import concourse.bass as bass
import concourse.bacc as bacc
import concourse.tile as tile
import concourse.mybir as mybir
from concourse import bass_utils
from concourse.tile_rust import add_dep_helper

B = 256
T = 256
H = 1024
NCORES = 8
SL = 128          # hidden slice per core
NIN = 81
KT = 11           # conv time taps
M3 = 3 * SL       # 384 rows per core (r|z|n)

f32 = mybir.dt.float32
bf16 = mybir.dt.bfloat16
int8 = mybir.dt.int8
AF = mybir.ActivationFunctionType
bfnp = ml_dtypes.bfloat16

_nc_cache = {}


def _build(t_steps):
    if t_steps in _nc_cache:
        return _nc_cache[t_steps]
    nc = bacc.Bacc("TRN2", target_bir_lowering=False, debug=False,
                   num_devices=NCORES)
    din = {}

    def inp(name, shape, dt=bf16):
        din[name] = nc.dram_tensor(name, shape, dt, kind="ExternalInput").ap()

    inp('xpad', [t_steps + 10, NIN, B])
    inp('m1t', [NIN, KT * M3])
    inp('w2xt', [NIN, M3])
    inp('w3xt', [NIN, M3])
    for nm in ('whh0t', 'weff2t', 'whh1t', 'w3h1t', 'whh2t'):
        inp(nm, [128, 8 * M3])
    inp('ft', [128, 3 * NIN])
    for nm in ('xc1', 'xc2', 'xc3'):
        inp(nm, [128, 3 * B])
    inp('ident', [128, 128])
    for nm in ('h1t0', 'h2t0', 'h3t0'):
        inp(nm, [128, 8 * B])
    for nm in ('h1own0', 'h2own0', 'h3own0'):
        inp(nm, [128, B], f32)
    ychunk = t_steps * NIN * B // NCORES
    yout = nc.dram_tensor('yout', [ychunk], f32, kind="ExternalOutput").ap()

    with tile.TileContext(nc) as tc:
        with tc.tile_pool(name="wpool", bufs=1) as wp, \
             tc.tile_pool(name="hpool", bufs=2) as hp, \
             tc.tile_pool(name="xwpool", bufs=14) as xp, \
             tc.tile_pool(name="scratch", bufs=1) as sp, \
             tc.tile_pool(name="ownpool", bufs=2) as op_, \
             tc.tile_pool(name="pspool", bufs=1, space="PSUM") as pp, \
             tc.tile_pool(name="drampool", bufs=2, space="DRAM") as dp:

            # ---- load constants ----
            W = {}
            for nm, shape in (('m1t', [NIN, KT * M3]), ('w2xt', [NIN, M3]),
                              ('w3xt', [NIN, M3]), ('whh0t', [128, 8 * M3]),
                              ('weff2t', [128, 8 * M3]), ('whh1t', [128, 8 * M3]),
                              ('w3h1t', [128, 8 * M3]), ('whh2t', [128, 8 * M3]),
                              ('ft', [128, 3 * NIN])):
                w_t = wp.tile(shape, bf16, tag=nm, name=nm + '_sb')
                nc.sync.dma_start(out=w_t[:], in_=din[nm][:])
                W[nm] = w_t
            XC = {}
            for nm in ('xc1', 'xc2', 'xc3'):
                c_t = wp.tile([128, 3 * B], bf16, tag=nm, name=nm + '_sb')
                nc.sync.dma_start(out=c_t[:], in_=din[nm][:])
                XC[nm] = c_t
            ident = wp.tile([128, 128], bf16, tag='ident', name='ident_sb')
            nc.sync.dma_start(out=ident[:], in_=din['ident'][:])

            hT = {}
            h1t_i = hp.tile([128, 8 * B], bf16, tag='h1T', name='h1T_i')
            nc.sync.dma_start(out=h1t_i[:], in_=din['h1t0'][:])
            hT[1] = h1t_i
            h23_i = hp.tile([128, 16 * B], bf16, tag='h23T', name='h23T_i')
            nc.sync.dma_start(out=h23_i[:, :8 * B], in_=din['h2t0'][:])
            nc.sync.dma_start(out=h23_i[:, 8 * B:], in_=din['h3t0'][:])
            hT[2], hT[3] = h23_i[:, :8 * B], h23_i[:, 8 * B:]
            hown = {}
            for li, nm in ((1, 'h1own0'), (2, 'h2own0'), (3, 'h3own0')):
                h_t = op_.tile([128, B], f32, tag=f'h{li}own', name=f'h{li}own_i')
                nc.sync.dma_start(out=h_t[:], in_=din[nm][:])
                hown[li] = h_t
            hbf = {}   # bf16 images of 127*h (for local y matmuls)
            hq = {}    # int8 quantized 127*h (AllGather transport)
            h1gs = {}  # per-step h1 gather tiles (consumed by l23)
            h1bfs = {}  # per-step h1 bf16 slices (consumed by l23's y matmul)

            ypart = dp.tile([t_steps, NIN, B], f32, tag='ypart', name='ypart',
                            bufs=1)

            # ---- x window ring ----
            xw = {}

            def load_xw(j):
                x_t = xp.tile([NIN, B], bf16, tag='xw', name=f'xw{j}')
                nc.sync.dma_start(out=x_t[:], in_=din['xpad'][j])
                xw[j] = x_t

            for j in range(min(13, t_steps + 10)):
                load_xw(j)

            def mmgroup(dst, pairs, first=True, last=True, after=None):
                insts = []
                n = len(pairs)
                for idx, (lh, rh) in enumerate(pairs):
                    bi = nc.tensor.matmul(dst, lh, rh,
                                          start=(first and idx == 0),
                                          stop=(last and idx == n - 1))
                    insts.append(bi)
                if after is not None:
                    add_dep_helper(insts[0].ins, after[-1].ins,
                                   reason="psum bank group order")
                return insts

            def gru_gates(ps_r, ps_z, ps_xn, ps_hn, li, lname):
                """Gate math. State hown[li] holds 127*h (f32). Produces:
                hq[li] (int8, = round(127*h), for AllGather transport) and
                hbf[li] (bf16 image of 127*h, for local y matmuls).
                All weight matrices that consume h are pre-scaled by 1/127."""
                r = sp.tile([128, B], f32, tag=f'{lname}r', name=f'{lname}r')
                nc.scalar.activation(r[:], ps_r, AF.Sigmoid)
                z = sp.tile([128, B], f32, tag=f'{lname}z', name=f'{lname}z')
                nc.scalar.activation(z[:], ps_z, AF.Sigmoid)
                w = sp.tile([128, B], f32, tag=f'{lname}w', name=f'{lname}w')
                nc.vector.tensor_scalar(w[:], z[:], -1.0, 1.0,
                                        mybir.AluOpType.mult,
                                        mybir.AluOpType.add)
                u = sp.tile([128, B], f32, tag=f'{lname}u', name=f'{lname}u')
                nc.vector.tensor_mul(u[:], z[:], hown[li][:])
                t1 = sp.tile([128, B], f32, tag=f'{lname}t1', name=f'{lname}t1')
                nc.vector.tensor_mul(t1[:], r[:], ps_hn)
                t2 = sp.tile([128, B], f32, tag=f'{lname}t2', name=f'{lname}t2')
                nc.vector.tensor_add(t2[:], t1[:], ps_xn)
                n_t = sp.tile([128, B], f32, tag=f'{lname}n', name=f'{lname}n')
                nc.scalar.activation(n_t[:], t2[:], AF.Tanh)
                p = sp.tile([128, B], f32, tag=f'{lname}p', name=f'{lname}p')
                nc.vector.tensor_mul(p[:], n_t[:], w[:])
                h_q = op_.tile([128, B], int8, tag=f'{lname}q', name=f'{lname}q')
                nc.vector.scalar_tensor_tensor(
                    h_q[:], p[:], 127.0, u[:],
                    mybir.AluOpType.mult, mybir.AluOpType.add)
                hq[li] = h_q
                h_b = op_.tile([128, B], bf16, tag=f'{lname}bf', name=f'{lname}bf')
                nc.vector.scalar_tensor_tensor(
                    h_b[:], p[:], 127.0, u[:],
                    mybir.AluOpType.mult, mybir.AluOpType.add)
                hbf[li] = h_b
                h_new = op_.tile([128, B], f32, tag=f'{lname}own',
                                 name=f'{lname}own')
                nc.vector.scalar_tensor_tensor(
                    h_new[:], p[:], 127.0, u[:],
                    mybir.AluOpType.mult, mybir.AluOpType.add)
                hown[li] = h_new

            def l1_early(t, st):
                """x-side matmuls of layer-1 step t (no AG dependencies)."""
                j = t + 12
                if j < t_steps + 10:
                    load_xw(j)
                psA = pp.tile([128, 2 * B], f32, tag='psA1', name='psA1')
                psB = pp.tile([128, 2 * B], f32, tag='psB1', name='psB1')
                st['ps'] = (psA, psB)
                m1 = W['m1t']

                def m1_pairs(g):
                    return [(m1[:, kt * M3 + g * SL: kt * M3 + (g + 1) * SL],
                             xw[t + kt][:]) for kt in range(KT)]

                xc = XC['xc1']
                st['g_xn'] = mmgroup(psB[:, :B],
                                     [(ident[:], xc[:, 2 * B:3 * B])]
                                     + m1_pairs(2))
                st['g_rx'] = mmgroup(psA[:, :B],
                                     [(ident[:], xc[:, 0:B])] + m1_pairs(0),
                                     last=False)
                st['g_zx'] = mmgroup(psB[:, B:2 * B],
                                     [(ident[:], xc[:, B:2 * B])] + m1_pairs(1),
                                     last=False, after=st['g_xn'])

            def l1_late(t, st):
                """h-side matmuls + gates of layer-1 step t; issues AG1(t)."""
                psA, psB = st['ps']
                h1c = hT[1]

                def whh_pairs(g):
                    return [(W['whh0t'][:, k * M3 + g * SL: k * M3 + (g + 1) * SL],
                             h1c[:, k * B:(k + 1) * B]) for k in range(8)]

                g_rh = mmgroup(psA[:, :B], whh_pairs(0), first=False)
                mmgroup(psB[:, B:2 * B], whh_pairs(1), first=False)
                mmgroup(psA[:, B:2 * B], whh_pairs(2), after=g_rh)

                gru_gates(psA[:, :B], psB[:, B:2 * B], psB[:, :B],
                          psA[:, B:2 * B], 1, 'L1')

                agin1 = dp.tile([128, B], int8, tag='agin1', name='agin1', bufs=2)
                nc.sync.dma_start(out=agin1[:], in_=hq[1][:])
                agout1 = dp.tile([NCORES, 128, B], int8, tag='agout1',
                                 name='agout1', addr_space="Shared", bufs=2)
                nc.gpsimd.collective_compute(
                    "AllGather", mybir.AluOpType.bypass,
                    replica_groups=[list(range(NCORES))],
                    ins=[agin1[:].opt()], outs=[agout1[:].opt()])
                h1g = hp.tile([128, 8 * B], bf16, tag='h1T', name='h1T')
                nc.gpsimd.dma_start(out=h1g[:],
                                    in_=agout1[:].rearrange("k p b -> p k b"))
                hT[1] = h1g
                h1gs[t] = h1g
                h1bfs[t] = hbf[1]

            L23 = (
                (2, 'w2xt', 'whh1t', 'weff2t', 'xc2', 'L2'),
                (3, 'w3xt', 'whh2t', 'w3h1t', 'xc3', 'L3'))

            def l23_head(t, st):
                """Early (AG-independent) matmuls of layers 2/3 of step t:
                w2x + whh parts of the r and z gates."""
                for li, wx, whh, weff, xc, lname in L23:
                    hc = hT[li]
                    psA = pp.tile([128, 2 * B], f32, tag='psA23',
                                  name=f'psA{li}', bufs=2)
                    psB = pp.tile([128, 2 * B], f32, tag='psB23',
                                  name=f'psB{li}', bufs=2)
                    st[li] = (psA, psB)
                    for g, dst in ((0, psA[:, :B]), (1, psB[:, :B])):
                        pairs = [(ident[:], XC[xc][:, g * B:(g + 1) * B]),
                                 (W[wx][:, g * SL:(g + 1) * SL], xw[t + 5][:])]
                        pairs += [(W[whh][:, k * M3 + g * SL: k * M3 + (g + 1) * SL],
                                   hc[:, k * B:(k + 1) * B]) for k in range(8)]
                        mmgroup(dst, pairs, last=False)

            def l23_tail(t, st):
                """Late matmuls (weff @ h1[t], hn, xn) + gates + AG23 + y."""
                h1c = h1gs.pop(t)
                h1b = h1bfs.pop(t)
                for li, wx, whh, weff, xc, lname in L23:
                    hc = hT[li]
                    psA, psB = st[li]

                    def weff_pairs(g):
                        return [(W[weff][:, k * M3 + g * SL: k * M3 + (g + 1) * SL],
                                 h1c[:, k * B:(k + 1) * B]) for k in range(8)]

                    g_rl = mmgroup(psA[:, :B], weff_pairs(0), first=False)
                    g_zl = mmgroup(psB[:, :B], weff_pairs(1), first=False)
                    mmgroup(psA[:, B:2 * B],
                            [(W[whh][:, k * M3 + 2 * SL: k * M3 + 3 * SL],
                              hc[:, k * B:(k + 1) * B]) for k in range(8)],
                            after=g_rl)
                    mmgroup(psB[:, B:2 * B],
                            [(ident[:], XC[xc][:, 2 * B:3 * B]),
                             (W[wx][:, 2 * SL:3 * SL], xw[t + 5][:])]
                            + weff_pairs(2), after=g_zl)
                    gru_gates(psA[:, :B], psB[:, :B], psB[:, B:2 * B],
                              psA[:, B:2 * B], li, lname)

                # AllGather h2 & h3 (skip after last step)
                if t + 1 < t_steps:
                    agin23 = dp.tile([2, 128, B], int8, tag='agin23',
                                     name='agin23', bufs=2)
                    nc.sync.dma_start(out=agin23[0], in_=hq[2][:])
                    nc.sync.dma_start(out=agin23[1], in_=hq[3][:])
                    agout23 = dp.tile([NCORES, 2, 128, B], int8, tag='agout23',
                                      name='agout23', addr_space="Shared", bufs=2)
                    nc.gpsimd.collective_compute(
                        "AllGather", mybir.AluOpType.bypass,
                        replica_groups=[list(range(NCORES))],
                        ins=[agin23[:].opt()], outs=[agout23[:].opt()])
                    h23g = hp.tile([128, 16 * B], bf16, tag='h23T', name='h23T')
                    nc.gpsimd.dma_start(
                        out=h23g[:, :8 * B],
                        in_=agout23[:, 0].rearrange("k p b -> p k b"))
                    nc.gpsimd.dma_start(
                        out=h23g[:, 8 * B:],
                        in_=agout23[:, 1].rearrange("k p b -> p k b"))
                    hT[2], hT[3] = h23g[:, :8 * B], h23g[:, 8 * B:]

                # y partials from own slices
                ps_y = pp.tile([NIN, B], f32, tag='yps', name='yps')
                nc.tensor.matmul(ps_y[:], W['ft'][:, 0:NIN], h1b[:],
                                 start=True, stop=False)
                nc.tensor.matmul(ps_y[:], W['ft'][:, NIN:2 * NIN], hbf[2][:],
                                 start=False, stop=False)
                nc.tensor.matmul(ps_y[:], W['ft'][:, 2 * NIN:3 * NIN], hbf[3][:],
                                 start=False, stop=True)
                ysb = sp.tile([NIN, B], f32, tag='ysb', name='ysb')
                nc.vector.tensor_copy(ysb[:], ps_y[:])
                nc.sync.dma_start(out=ypart[t], in_=ysb[:])
                xw.pop(t - 1, None)

            # ---- software-pipelined loop ----
            st1 = {}
            l1_early(0, st1)
            l1_late(0, st1)
            for t in range(t_steps):
                st23 = {}
                if t + 1 < t_steps:
                    st1 = {}
                    l1_early(t + 1, st1)
                l23_head(t, st23)
                l23_tail(t, st23)
                if t + 1 < t_steps:
                    l1_late(t + 1, st1)

            # ---- final ReduceScatter of y partials ----
            yred = dp.tile([t_steps * NIN * B // NCORES], f32, tag='yred',
                           name='yred', bufs=1)
            nc.gpsimd.collective_compute(
                "ReduceScatter", mybir.AluOpType.add,
                replica_groups=[list(range(NCORES))],
                ins=[ypart[:].opt()], outs=[yred[:].opt()])
            nc.sync.dma_start(out=yout[:], in_=yred[:])

    nc.compile()
    _nc_cache[t_steps] = nc
    return nc


def _prepare(x, cond, h1, h2, h3, params, t_steps):
    """Host-side folding. Returns (in_maps, yb)."""
    p = params
    fp = np.float32

    def A(v):
        return np.ascontiguousarray(np.asarray(v), dtype=fp)

    def BF(v):
        return np.ascontiguousarray(np.asarray(v, dtype=fp)).astype(bfnp)

    Wih0, Whh0 = A(p['Wih0']), A(p['Whh0'])
    Wih1, Whh1 = A(p['Wih1']), A(p['Whh1'])
    Wih2, Whh2 = A(p['Wih2']), A(p['Whh2'])
    conv_w = A(p['conv_w'])[:, 0]          # [64, 21, 11]
    conv_b = A(p['conv_b'])
    cond_np = A(cond)

    cond1 = cond_np @ A(p['cond0_w']).T + A(p['cond0_b'])
    cond2 = cond_np @ A(p['cond1_w']).T + A(p['cond1_b'])
    cond3 = cond_np @ A(p['cond2_w']).T + A(p['cond2_b'])

    # --- M1 fold: conv + Wih0 ---
    Wr = Wih0[:, :1984].reshape(3 * H, 64, 31)
    M1 = np.zeros((3 * H, KT, NIN), np.float32)
    for kf in range(21):
        tmp = np.einsum('rcf,ck->rfk', Wr, conv_w[:, kf, :], optimize=True)
        fins = 2 * np.arange(31) + kf
        M1[:, :, fins] += tmp.transpose(0, 2, 1)
    c1 = Wih0[:, :1984] @ np.repeat(conv_b, 31)
    xc1 = Wih0[:, 1984:] @ cond1.T + c1[:, None]

    W2x = Wih1[:, :H] @ A(p['in1_w'])
    Weff2 = Wih1[:, :H] @ A(p['ht0_w'])
    xc2 = (Wih1[:, :H] @ (A(p['in1_b']) + A(p['ht0_b'])))[:, None] \
        + Wih1[:, H:] @ cond2.T

    W3x = Wih2[:, :H] @ A(p['in2_w'])
    W3h1 = Wih2[:, :H] @ (A(p['ht1_w']) + A(p['ht2_w']) @ A(p['ht0_w']))
    xc3 = (Wih2[:, :H] @ (A(p['in2_b']) + A(p['ht1_b']) + A(p['ht2_b'])
                          + A(p['ht2_w']) @ A(p['ht0_b'])))[:, None] \
        + Wih2[:, H:] @ cond3.T

    F1 = A(p['final_w']) @ A(p['out0_w'])
    F2 = A(p['final_w']) @ A(p['out1_w'])
    F3 = A(p['final_w']) @ A(p['out2_w'])
    yb = A(p['final_w']) @ (A(p['out0_b']) + A(p['out1_b'])
                            + A(p['out2_b'])) + A(p['final_b'])

    # --- x: [B,1,81,T] -> padded [T+10, 81, B] ---
    xs = A(x)[:, 0, :, :t_steps]                      # [B, 81, t]
    xpad = np.zeros((t_steps + 10, NIN, B), np.float32)
    xpad[5:5 + t_steps] = xs.transpose(2, 1, 0)
    xpad = xpad.astype(bfnp)

    def hmaj(h):
        return (127.0 * A(h).T.reshape(8, 128, B).transpose(1, 0, 2)
                .reshape(128, 8 * B)).astype(bfnp)

    h1T, h2T, h3T = hmaj(h1), hmaj(h2), hmaj(h3)

    def kmaj(w):          # [384, 1024] -> lhsT sbuf layout [128, 8*384]
        return np.ascontiguousarray(
            w.T.reshape(8, 128, M3).transpose(1, 0, 2).reshape(128, 8 * M3)
        ).astype(bfnp)

    in_maps = []
    for i in range(NCORES):
        idx = np.concatenate([np.arange(g * H + i * SL, g * H + (i + 1) * SL)
                              for g in range(3)])
        m1t = np.ascontiguousarray(
            M1[idx].transpose(2, 1, 0).reshape(NIN, KT * M3)).astype(bfnp)
        im = {
            'xpad': xpad,
            'm1t': m1t,
            'w2xt': BF(W2x[idx].T),
            'w3xt': BF(W3x[idx].T),
            'whh0t': kmaj(Whh0[idx] / 127.0),
            'weff2t': kmaj(Weff2[idx] / 127.0),
            'whh1t': kmaj(Whh1[idx] / 127.0),
            'w3h1t': kmaj(W3h1[idx] / 127.0),
            'whh2t': kmaj(Whh2[idx] / 127.0),
            'ft': BF(np.concatenate(
                [F1[:, i * SL:(i + 1) * SL].T,
                 F2[:, i * SL:(i + 1) * SL].T,
                 F3[:, i * SL:(i + 1) * SL].T], axis=1) / 127.0),
            'xc1': np.ascontiguousarray(
                xc1[idx].reshape(3, SL, B).transpose(1, 0, 2)
                .reshape(SL, 3 * B)).astype(bfnp),
            'xc2': np.ascontiguousarray(
                xc2[idx].reshape(3, SL, B).transpose(1, 0, 2)
                .reshape(SL, 3 * B)).astype(bfnp),
            'xc3': np.ascontiguousarray(
                xc3[idx].reshape(3, SL, B).transpose(1, 0, 2)
                .reshape(SL, 3 * B)).astype(bfnp),
            'ident': np.eye(128, dtype=np.float32).astype(bfnp),
            'h1t0': h1T, 'h2t0': h2T, 'h3t0': h3T,
            'h1own0': np.ascontiguousarray(127.0 * A(h1).T[i * SL:(i + 1) * SL]),
            'h2own0': np.ascontiguousarray(127.0 * A(h2).T[i * SL:(i + 1) * SL]),
            'h3own0': np.ascontiguousarray(127.0 * A(h3).T[i * SL:(i + 1) * SL]),
        }
        in_maps.append(im)
    return in_maps, yb


def _run(x, cond, h1, h2, h3, params, t_steps=T, trace=False):
    nc = _build(t_steps)
    in_maps, yb = _prepare(x, cond, h1, h2, h3, params, t_steps)
    res = bass_utils.run_bass_kernel_spmd(
        nc, in_maps, core_ids=list(range(NCORES)), trace=trace)
    chunks = [res.results[i]['yout'] for i in range(NCORES)]
    y = np.concatenate(chunks).reshape(t_steps, NIN, B).transpose(0, 2, 1)
    y = y + yb[None, None, :]
    return np.ascontiguousarray(y, dtype=np.float32), res


def kernel(x, cond, h1, h2, h3, params):
    y, _ = _run(x, cond, h1, h2, h3, params)
    return y


# revision 17
# speedup vs baseline: 1.2879x; 1.1678x over previous
"""Trainium2 Bass kernel for nn_RNNModelWithConditioning.

Strategy: 8-way model-parallel over the hidden dimension (each core owns a
128-row slice of every gate block), full batch per core (N=256). All
weights stay SBUF-resident across the T=256 recurrence. Host-side numpy
folds the conv + input-side linears + cross-layer links into per-step
matmul-only forms:

  gx1[t] = sum_kt M1[:,kt,:] @ xpad[t+kt]          (conv+Wih0 folded)
  gx2[t] = W2x @ x0[t] + Weff2 @ h1[t] + xc2       (in1/ht0 folded)
  gx3[t] = W3x @ x0[t] + W3h1 @ h1[t] + xc3        (in2/ht1/ht2/ht0 folded)
  y[t]   = F1 @ h1[t] + F2 @ h2[t] + F3 @ h3[t] + yb

Matmul operands are bf16 (weights, x windows, gathered h); the local GRU
state h_own is kept in f32 so rounding does not compound step to step.
The loop is software-pipelined: layer 1 of step t+1 is emitted before
layers 2/3 of step t, so each AllGather's round trip hides under the
other half-step's matmuls. y partials (feature-sliced) accumulate in DRAM
and are ReduceScattered at the end; the host concatenates the 8 chunks.
"""
import numpy as np
import ml_dtypes

import concourse.bass as bass
import concourse.bacc as bacc
import concourse.tile as tile
import concourse.mybir as mybir
from concourse import bass_utils
from concourse.tile_rust import add_dep_helper

B = 256
T = 256
H = 1024
NCORES = 8
SL = 128          # hidden slice per core
NIN = 81
KT = 11           # conv time taps
M3 = 3 * SL       # 384 rows per core (r|z|n)

f32 = mybir.dt.float32
bf16 = mybir.dt.bfloat16
int8 = mybir.dt.int8
AF = mybir.ActivationFunctionType
bfnp = ml_dtypes.bfloat16

_nc_cache = {}


def _build(t_steps):
    if t_steps in _nc_cache:
        return _nc_cache[t_steps]
    nc = bacc.Bacc("TRN2", target_bir_lowering=False, debug=False,
                   num_devices=NCORES)
    din = {}

    def inp(name, shape, dt=bf16):
        din[name] = nc.dram_tensor(name, shape, dt, kind="ExternalInput").ap()

    inp('xpad', [t_steps + 10, NIN, B])
    inp('m1t', [NIN, KT * M3])
    inp('w2xt', [NIN, M3])
    inp('w3xt', [NIN, M3])
    for nm in ('whh0t', 'weff2t', 'whh1t', 'w3h1t', 'whh2t'):
        inp(nm, [128, 8 * M3])
    inp('ft', [128, 3 * NIN])
    for nm in ('xc1', 'xc2', 'xc3'):
        inp(nm, [128, 3 * B])
    inp('ident', [128, 128])
    for nm in ('h1t0', 'h2t0', 'h3t0'):
        inp(nm, [128, 8 * B])
    for nm in ('h1own0', 'h2own0', 'h3own0'):
        inp(nm, [128, B], f32)
    ychunk = t_steps * NIN * B // NCORES
    yout = nc.dram_tensor('yout', [ychunk], f32, kind="ExternalOutput").ap()

    with tile.TileContext(nc) as tc:
        with tc.tile_pool(name="wpool", bufs=1) as wp, \
             tc.tile_pool(name="hpool", bufs=2) as hp, \
             tc.tile_pool(name="xwpool", bufs=14) as xp, \
             tc.tile_pool(name="scratch", bufs=1) as sp, \
             tc.tile_pool(name="ownpool", bufs=2) as op_, \
             tc.tile_pool(name="pspool", bufs=1, space="PSUM") as pp, \
             tc.tile_pool(name="drampool", bufs=2, space="DRAM") as dp:

            # ---- load constants ----
            W = {}
            for nm, shape in (('m1t', [NIN, KT * M3]), ('w2xt', [NIN, M3]),
                              ('w3xt', [NIN, M3]), ('whh0t', [128, 8 * M3]),
                              ('weff2t', [128, 8 * M3]), ('whh1t', [128, 8 * M3]),
                              ('w3h1t', [128, 8 * M3]), ('whh2t', [128, 8 * M3]),
                              ('ft', [128, 3 * NIN])):
                w_t = wp.tile(shape, bf16, tag=nm, name=nm + '_sb')
                nc.sync.dma_start(out=w_t[:], in_=din[nm][:])
                W[nm] = w_t
            XC = {}
            for nm in ('xc1', 'xc2', 'xc3'):
                c_t = wp.tile([128, 3 * B], bf16, tag=nm, name=nm + '_sb')
                nc.sync.dma_start(out=c_t[:], in_=din[nm][:])
                XC[nm] = c_t
            ident = wp.tile([128, 128], bf16, tag='ident', name='ident_sb')
            nc.sync.dma_start(out=ident[:], in_=din['ident'][:])

            hT = {}
            h1t_i = hp.tile([128, 8 * B], bf16, tag='h1T', name='h1T_i')
            nc.sync.dma_start(out=h1t_i[:], in_=din['h1t0'][:])
            hT[1] = h1t_i
            h23_i = hp.tile([128, 16 * B], bf16, tag='h23T', name='h23T_i')
            nc.sync.dma_start(out=h23_i[:, :8 * B], in_=din['h2t0'][:])
            nc.sync.dma_start(out=h23_i[:, 8 * B:], in_=din['h3t0'][:])
            hT[2], hT[3] = h23_i[:, :8 * B], h23_i[:, 8 * B:]
            hown = {}
            for li, nm in ((1, 'h1own0'), (2, 'h2own0'), (3, 'h3own0')):
                h_t = op_.tile([128, B], f32, tag=f'h{li}own', name=f'h{li}own_i')
                nc.sync.dma_start(out=h_t[:], in_=din[nm][:])
                hown[li] = h_t
            hbf = {}   # bf16 images of 127*h (for local y matmuls)
            hq = {}    # int8 quantized 127*h (AllGather transport)
            h1gs = {}  # per-step h1 gather tiles (consumed by l23)
            h1bfs = {}  # per-step h1 bf16 slices (consumed by l23's y matmul)

            ypart = dp.tile([t_steps, NIN, B], f32, tag='ypart', name='ypart',
                            bufs=1)

            # ---- x window ring ----
            xw = {}

            def load_xw(j):
                x_t = xp.tile([NIN, B], bf16, tag='xw', name=f'xw{j}')
                nc.sync.dma_start(out=x_t[:], in_=din['xpad'][j])
                xw[j] = x_t

            for j in range(min(13, t_steps + 10)):
                load_xw(j)

            def mmgroup(dst, pairs, first=True, last=True, after=None):
                insts = []
                n = len(pairs)
                for idx, (lh, rh) in enumerate(pairs):
                    bi = nc.tensor.matmul(dst, lh, rh,
                                          start=(first and idx == 0),
                                          stop=(last and idx == n - 1))
                    insts.append(bi)
                if after is not None:
                    add_dep_helper(insts[0].ins, after[-1].ins,
                                   reason="psum bank group order")
                return insts

            def gru_gates(ps_r, ps_z, ps_xn, ps_hn, li, lname):
                """Gate math. State hown[li] holds 127*h (f32). Produces:
                hq[li] (int8, = round(127*h), for AllGather transport) and
                hbf[li] (bf16 image of 127*h, for local y matmuls).
                All weight matrices that consume h are pre-scaled by 1/127."""
                r = sp.tile([128, B], f32, tag=f'{lname}r', name=f'{lname}r')
                nc.scalar.activation(r[:], ps_r, AF.Sigmoid)
                z = sp.tile([128, B], f32, tag=f'{lname}z', name=f'{lname}z')
                nc.scalar.activation(z[:], ps_z, AF.Sigmoid)
                w = sp.tile([128, B], f32, tag=f'{lname}w', name=f'{lname}w')
                nc.vector.tensor_scalar(w[:], z[:], -1.0, 1.0,
                                        mybir.AluOpType.mult,
                                        mybir.AluOpType.add)
                u = sp.tile([128, B], f32, tag=f'{lname}u', name=f'{lname}u')
                nc.vector.tensor_mul(u[:], z[:], hown[li][:])
                t1 = sp.tile([128, B], f32, tag=f'{lname}t1', name=f'{lname}t1')
                nc.vector.tensor_mul(t1[:], r[:], ps_hn)
                t2 = sp.tile([128, B], f32, tag=f'{lname}t2', name=f'{lname}t2')
                nc.vector.tensor_add(t2[:], t1[:], ps_xn)
                n_t = sp.tile([128, B], f32, tag=f'{lname}n', name=f'{lname}n')
                nc.scalar.activation(n_t[:], t2[:], AF.Tanh)
                p = sp.tile([128, B], f32, tag=f'{lname}p', name=f'{lname}p')
                nc.vector.tensor_mul(p[:], n_t[:], w[:])
                h_q = op_.tile([128, B], int8, tag=f'{lname}q', name=f'{lname}q')
                nc.vector.scalar_tensor_tensor(
                    h_q[:], p[:], 127.0, u[:],
                    mybir.AluOpType.mult, mybir.AluOpType.add)
                hq[li] = h_q
                h_b = op_.tile([128, B], bf16, tag=f'{lname}bf', name=f'{lname}bf')
                nc.vector.scalar_tensor_tensor(
                    h_b[:], p[:], 127.0, u[:],
                    mybir.AluOpType.mult, mybir.AluOpType.add)
                hbf[li] = h_b
                h_new = op_.tile([128, B], f32, tag=f'{lname}own',
                                 name=f'{lname}own')
                nc.vector.scalar_tensor_tensor(
                    h_new[:], p[:], 127.0, u[:],
                    mybir.AluOpType.mult, mybir.AluOpType.add)
                hown[li] = h_new

            def l1_early(t, st):
                """x-side matmuls of layer-1 step t (no AG dependencies)."""
                j = t + 12
                if j < t_steps + 10:
                    load_xw(j)
                psA = pp.tile([128, 2 * B], f32, tag='psA1', name='psA1')
                psB = pp.tile([128, 2 * B], f32, tag='psB1', name='psB1')
                st['ps'] = (psA, psB)
                m1 = W['m1t']

                def m1_pairs(g):
                    return [(m1[:, kt * M3 + g * SL: kt * M3 + (g + 1) * SL],
                             xw[t + kt][:]) for kt in range(KT)]

                xc = XC['xc1']
                st['g_xn'] = mmgroup(psB[:, :B],
                                     [(ident[:], xc[:, 2 * B:3 * B])]
                                     + m1_pairs(2))
                st['g_rx'] = mmgroup(psA[:, :B],
                                     [(ident[:], xc[:, 0:B])] + m1_pairs(0),
                                     last=False)
                st['g_zx'] = mmgroup(psB[:, B:2 * B],
                                     [(ident[:], xc[:, B:2 * B])] + m1_pairs(1),
                                     last=False, after=st['g_xn'])

            def l1_late(t, st):
                """h-side matmuls + gates of layer-1 step t; issues AG1(t)."""
                psA, psB = st['ps']
                h1c = hT[1]

                def whh_pairs(g):
                    return [(W['whh0t'][:, k * M3 + g * SL: k * M3 + (g + 1) * SL],
                             h1c[:, k * B:(k + 1) * B]) for k in range(8)]

                g_rh = mmgroup(psA[:, :B], whh_pairs(0), first=False)
                mmgroup(psB[:, B:2 * B], whh_pairs(1), first=False)
                mmgroup(psA[:, B:2 * B], whh_pairs(2), after=g_rh)

                gru_gates(psA[:, :B], psB[:, B:2 * B], psB[:, :B],
                          psA[:, B:2 * B], 1, 'L1')

                agin1 = dp.tile([128, B], int8, tag='agin1', name='agin1', bufs=2)
                nc.sync.dma_start(out=agin1[:], in_=hq[1][:])
                agout1 = dp.tile([NCORES, 128, B], int8, tag='agout1',
                                 name='agout1', addr_space="Shared", bufs=2)
                nc.gpsimd.collective_compute(
                    "AllGather", mybir.AluOpType.bypass,
                    replica_groups=[list(range(NCORES))],
                    ins=[agin1[:].opt()], outs=[agout1[:].opt()])
                h1g = hp.tile([128, 8 * B], bf16, tag='h1T', name='h1T')
                nc.gpsimd.dma_start(out=h1g[:],
                                    in_=agout1[:].rearrange("k p b -> p k b"))
                hT[1] = h1g
                h1gs[t] = h1g
                h1bfs[t] = hbf[1]

            L23 = (
                (2, 'w2xt', 'whh1t', 'weff2t', 'xc2', 'L2'),
                (3, 'w3xt', 'whh2t', 'w3h1t', 'xc3', 'L3'))

            def l23_head(t, st):
                """Early (AG-independent) matmuls of layers 2/3 of step t:
                w2x + whh parts of the r and z gates."""
                for li, wx, whh, weff, xc, lname in L23:
                    hc = hT[li]
                    psA = pp.tile([128, 2 * B], f32, tag='psA23',
                                  name=f'psA{li}', bufs=2)
                    psB = pp.tile([128, 2 * B], f32, tag='psB23',
                                  name=f'psB{li}', bufs=2)
                    st[li] = (psA, psB)
                    for g, dst in ((0, psA[:, :B]), (1, psB[:, :B])):
                        pairs = [(ident[:], XC[xc][:, g * B:(g + 1) * B]),
                                 (W[wx][:, g * SL:(g + 1) * SL], xw[t + 5][:])]
                        pairs += [(W[whh][:, k * M3 + g * SL: k * M3 + (g + 1) * SL],
                                   hc[:, k * B:(k + 1) * B]) for k in range(8)]
                        mmgroup(dst, pairs, last=False)

            def l23_tail(t, st):
                """Late matmuls (weff @ h1[t], hn, xn) + gates + AG23 + y."""
                h1c = h1gs.pop(t)
                h1b = h1bfs.pop(t)
                for li, wx, whh, weff, xc, lname in L23:
                    hc = hT[li]
                    psA, psB = st[li]

                    def weff_pairs(g):
                        return [(W[weff][:, k * M3 + g * SL: k * M3 + (g + 1) * SL],
                                 h1c[:, k * B:(k + 1) * B]) for k in range(8)]

                    g_rl = mmgroup(psA[:, :B], weff_pairs(0), first=False)
                    g_zl = mmgroup(psB[:, :B], weff_pairs(1), first=False)
                    mmgroup(psA[:, B:2 * B],
                            [(W[whh][:, k * M3 + 2 * SL: k * M3 + 3 * SL],
                              hc[:, k * B:(k + 1) * B]) for k in range(8)],
                            after=g_rl)
                    mmgroup(psB[:, B:2 * B],
                            [(ident[:], XC[xc][:, 2 * B:3 * B]),
                             (W[wx][:, 2 * SL:3 * SL], xw[t + 5][:])]
                            + weff_pairs(2), after=g_zl)
                    gru_gates(psA[:, :B], psB[:, :B], psB[:, B:2 * B],
                              psA[:, B:2 * B], li, lname)

                # AllGather h2 & h3 (skip after last step)
                if t + 1 < t_steps:
                    agin23 = dp.tile([2, 128, B], int8, tag='agin23',
                                     name='agin23', bufs=2)
                    nc.sync.dma_start(out=agin23[0], in_=hq[2][:])
                    nc.sync.dma_start(out=agin23[1], in_=hq[3][:])
                    agout23 = dp.tile([NCORES, 2, 128, B], int8, tag='agout23',
                                      name='agout23', addr_space="Shared", bufs=2)
                    nc.gpsimd.collective_compute(
                        "AllGather", mybir.AluOpType.bypass,
                        replica_groups=[list(range(NCORES))],
                        ins=[agin23[:].opt()], outs=[agout23[:].opt()])
                    h23g = hp.tile([128, 16 * B], bf16, tag='h23T', name='h23T')
                    nc.gpsimd.dma_start(
                        out=h23g[:, :8 * B],
                        in_=agout23[:, 0].rearrange("k p b -> p k b"))
                    nc.gpsimd.dma_start(
                        out=h23g[:, 8 * B:],
                        in_=agout23[:, 1].rearrange("k p b -> p k b"))
                    hT[2], hT[3] = h23g[:, :8 * B], h23g[:, 8 * B:]

                # y partials from own slices
                ps_y = pp.tile([NIN, B], f32, tag='yps', name='yps')
                nc.tensor.matmul(ps_y[:], W['ft'][:, 0:NIN], h1b[:],
                                 start=True, stop=False)
                nc.tensor.matmul(ps_y[:], W['ft'][:, NIN:2 * NIN], hbf[2][:],
                                 start=False, stop=False)
                nc.tensor.matmul(ps_y[:], W['ft'][:, 2 * NIN:3 * NIN], hbf[3][:],
                                 start=False, stop=True)
                ysb = sp.tile([NIN, B], f32, tag='ysb', name='ysb')
                nc.vector.tensor_copy(ysb[:], ps_y[:])
                nc.sync.dma_start(out=ypart[t], in_=ysb[:])
                xw.pop(t - 1, None)

            # ---- software-pipelined loop ----
            st1 = {}
            l1_early(0, st1)
            l1_late(0, st1)
            for t in range(t_steps):
                if t + 1 < t_steps:
                    st1 = {}
                    l1_early(t + 1, st1)
                    l1_late(t + 1, st1)
                st23 = {}
                l23_head(t, st23)
                l23_tail(t, st23)

            # ---- final ReduceScatter of y partials ----
            yred = dp.tile([t_steps * NIN * B // NCORES], f32, tag='yred',
                           name='yred', bufs=1)
            nc.gpsimd.collective_compute(
                "ReduceScatter", mybir.AluOpType.add,
                replica_groups=[list(range(NCORES))],
                ins=[ypart[:].opt()], outs=[yred[:].opt()])
            nc.sync.dma_start(out=yout[:], in_=yred[:])

    nc.compile()
    _nc_cache[t_steps] = nc
    return nc


def _prepare(x, cond, h1, h2, h3, params, t_steps):
    """Host-side folding. Returns (in_maps, yb)."""
    p = params
    fp = np.float32

    def A(v):
        return np.ascontiguousarray(np.asarray(v), dtype=fp)

    def BF(v):
        return np.ascontiguousarray(np.asarray(v, dtype=fp)).astype(bfnp)

    Wih0, Whh0 = A(p['Wih0']), A(p['Whh0'])
    Wih1, Whh1 = A(p['Wih1']), A(p['Whh1'])
    Wih2, Whh2 = A(p['Wih2']), A(p['Whh2'])
    conv_w = A(p['conv_w'])[:, 0]          # [64, 21, 11]
    conv_b = A(p['conv_b'])
    cond_np = A(cond)

    cond1 = cond_np @ A(p['cond0_w']).T + A(p['cond0_b'])
    cond2 = cond_np @ A(p['cond1_w']).T + A(p['cond1_b'])
    cond3 = cond_np @ A(p['cond2_w']).T + A(p['cond2_b'])

    # --- M1 fold: conv + Wih0 ---
    Wr = Wih0[:, :1984].reshape(3 * H, 64, 31)
    M1 = np.zeros((3 * H, KT, NIN), np.float32)
    for kf in range(21):
        tmp = np.einsum('rcf,ck->rfk', Wr, conv_w[:, kf, :], optimize=True)
        fins = 2 * np.arange(31) + kf
        M1[:, :, fins] += tmp.transpose(0, 2, 1)
    c1 = Wih0[:, :1984] @ np.repeat(conv_b, 31)
    xc1 = Wih0[:, 1984:] @ cond1.T + c1[:, None]

    W2x = Wih1[:, :H] @ A(p['in1_w'])
    Weff2 = Wih1[:, :H] @ A(p['ht0_w'])
    xc2 = (Wih1[:, :H] @ (A(p['in1_b']) + A(p['ht0_b'])))[:, None] \
        + Wih1[:, H:] @ cond2.T

    W3x = Wih2[:, :H] @ A(p['in2_w'])
    W3h1 = Wih2[:, :H] @ (A(p['ht1_w']) + A(p['ht2_w']) @ A(p['ht0_w']))
    xc3 = (Wih2[:, :H] @ (A(p['in2_b']) + A(p['ht1_b']) + A(p['ht2_b'])
                          + A(p['ht2_w']) @ A(p['ht0_b'])))[:, None] \
        + Wih2[:, H:] @ cond3.T

    F1 = A(p['final_w']) @ A(p['out0_w'])
    F2 = A(p['final_w']) @ A(p['out1_w'])
    F3 = A(p['final_w']) @ A(p['out2_w'])
    yb = A(p['final_w']) @ (A(p['out0_b']) + A(p['out1_b'])
                            + A(p['out2_b'])) + A(p['final_b'])

    # --- x: [B,1,81,T] -> padded [T+10, 81, B] ---
    xs = A(x)[:, 0, :, :t_steps]                      # [B, 81, t]
    xpad = np.zeros((t_steps + 10, NIN, B), np.float32)
    xpad[5:5 + t_steps] = xs.transpose(2, 1, 0)
    xpad = xpad.astype(bfnp)

    def hmaj(h):
        return (127.0 * A(h).T.reshape(8, 128, B).transpose(1, 0, 2)
                .reshape(128, 8 * B)).astype(bfnp)

    h1T, h2T, h3T = hmaj(h1), hmaj(h2), hmaj(h3)

    def kmaj(w):          # [384, 1024] -> lhsT sbuf layout [128, 8*384]
        return np.ascontiguousarray(
            w.T.reshape(8, 128, M3).transpose(1, 0, 2).reshape(128, 8 * M3)
        ).astype(bfnp)

    in_maps = []
    for i in range(NCORES):
        idx = np.concatenate([np.arange(g * H + i * SL, g * H + (i + 1) * SL)
                              for g in range(3)])
        m1t = np.ascontiguousarray(
            M1[idx].transpose(2, 1, 0).reshape(NIN, KT * M3)).astype(bfnp)
        im = {
            'xpad': xpad,
            'm1t': m1t,
            'w2xt': BF(W2x[idx].T),
            'w3xt': BF(W3x[idx].T),
            'whh0t': kmaj(Whh0[idx] / 127.0),
            'weff2t': kmaj(Weff2[idx] / 127.0),
            'whh1t': kmaj(Whh1[idx] / 127.0),
            'w3h1t': kmaj(W3h1[idx] / 127.0),
            'whh2t': kmaj(Whh2[idx] / 127.0),
            'ft': BF(np.concatenate(
                [F1[:, i * SL:(i + 1) * SL].T,
                 F2[:, i * SL:(i + 1) * SL].T,
                 F3[:, i * SL:(i + 1) * SL].T], axis=1) / 127.0),
            'xc1': np.ascontiguousarray(
                xc1[idx].reshape(3, SL, B).transpose(1, 0, 2)
                .reshape(SL, 3 * B)).astype(bfnp),
            'xc2': np.ascontiguousarray(
                xc2[idx].reshape(3, SL, B).transpose(1, 0, 2)
                .reshape(SL, 3 * B)).astype(bfnp),
            'xc3': np.ascontiguousarray(
                xc3[idx].reshape(3, SL, B).transpose(1, 0, 2)
                .reshape(SL, 3 * B)).astype(bfnp),
            'ident': np.eye(128, dtype=np.float32).astype(bfnp),
            'h1t0': h1T, 'h2t0': h2T, 'h3t0': h3T,
            'h1own0': np.ascontiguousarray(127.0 * A(h1).T[i * SL:(i + 1) * SL]),
            'h2own0': np.ascontiguousarray(127.0 * A(h2).T[i * SL:(i + 1) * SL]),
            'h3own0': np.ascontiguousarray(127.0 * A(h3).T[i * SL:(i + 1) * SL]),
        }
        in_maps.append(im)
    return in_maps, yb


def _run(x, cond, h1, h2, h3, params, t_steps=T, trace=False):
    nc = _build(t_steps)
    in_maps, yb = _prepare(x, cond, h1, h2, h3, params, t_steps)
    res = bass_utils.run_bass_kernel_spmd(
        nc, in_maps, core_ids=list(range(NCORES)), trace=trace)
    chunks = [res.results[i]['yout'] for i in range(NCORES)]
    y = np.concatenate(chunks).reshape(t_steps, NIN, B).transpose(0, 2, 1)
    y = y + yb[None, None, :]
    return np.ascontiguousarray(y, dtype=np.float32), res


def kernel(x, cond, h1, h2, h3, params):
    y, _ = _run(x, cond, h1, h2, h3, params)
    return y
